# revision 7
# baseline (speedup 1.0000x reference)
"""Trainium2 Bass kernel for nn_BasicAttentionModel (3-layer GAT + edge MLP).

Fused single-launch design (8-core SPMD, dst-partitioned edges):
  - One Bass program holds all four phases (GAT x3 + edge MLP); intermediate
    node tables never leave the device.  Each core owns 98 consecutive
    128-node tiles (its dst range) and processes only its own edges; the
    per-layer node tables [prev+b | al_s | al_d] are computed in the edge
    phase epilogue (transpose + matmul with an augmented weight) and
    AllGathered across the 8 cores between phases.
  - Edges (with self-loops) are dst-sorted into tiles and grouped into 4
    src-chunk sub-tiles (int16 gather indices); the sub-tile size SUB is
    sized from the actual max chunk occupancy, rounded up to 128.
  - The edge MLP reuses the same slot layout (self-loop slots discarded on
    the host); edge_attr ships as 10-col bf16 in slot order; gather indices
    ship once (16 rows per tile) and are replicated across partitions on
    device.
  - The runner overlaps program build + jit trace + neuronx compile (worker
    thread) with host-side edge sorting and per-device input streaming
    (main thread).
"""
import threading
import numpy as np
import ml_dtypes

import concourse.bacc as bacc
import concourse.bass as bass
import concourse.mybir as mybir
import concourse.tile as tile
from concourse.bass_utils import run_bass_kernel_spmd
from concourse.masks import make_identity

F32 = mybir.dt.float32
BF16 = mybir.dt.bfloat16
I16 = mybir.dt.int16

N = 100000
E = 1600000
H = 8
CORES = 8
NP = 100352          # 784 * 128, divisible by 4 chunks of 25088
CH = 25088
TILE_N = 128
SUBS = 4
TILES = NP // TILE_N          # 784
TPC = TILES // CORES          # 98
OWN = TPC * TILE_N            # 12544 nodes per core
NODE_CH = 1792                # node-phase trip (OWN = 7 * 1792)
TW = 64                       # table row width (floats) = 256B


# ------------------------------------------------------------ host prep
def _sort_edges(src, dst, SUB):
    """Vectorized dst-tile / src-chunk slotting.
    Returns idx (int16, [TILES,16,SUBS*SUB/16]), dloc (f32
    [TILES,128,GROUPS]), slot_of (edge -> global slot)."""
    SLOTS = SUBS * SUB
    GROUPS = SLOTS // 128
    tile_i = (dst >> 7).astype(np.int64)
    chunk = src // CH
    key = tile_i * SUBS + chunk
    order = np.argsort(key, kind="stable")
    ks = key[order]
    starts = np.searchsorted(ks, np.arange(TILES * SUBS))
    rank = np.arange(len(ks)) - starts[ks]
    slot_sorted = tile_i[order] * SLOTS + chunk[order] * SUB + rank
    slot_of = np.empty(len(ks), np.int64)
    slot_of[order] = slot_sorted

    idx_flat = np.zeros(TILES * SLOTS, np.int16)
    idx_flat[slot_sorted] = (src[order] - chunk[order] * CH).astype(np.int16)
    dloc_flat = np.full(TILES * SLOTS, 255, np.int16)
    dloc_flat[slot_sorted] = (dst[order] - tile_i[order] * TILE_N).astype(np.int16)

    # wrap idx for dma_gather: j -> partition j%16, col j//16 (16 rows/tile;
    # replication to 128 partitions happens on device)
    w = idx_flat.reshape(TILES, SUBS, SUB // 16, 16)
    idx_w = np.transpose(w, (0, 3, 1, 2)).reshape(TILES, 16, SUBS * (SUB // 16))
    # dloc arranged [TILES, 128, GROUPS]: slot = g*128+p
    dl = dloc_flat.reshape(TILES, GROUPS, 128).transpose(0, 2, 1).copy()
    return idx_w, dl, slot_of


# ------------------------------------------------------------ program
def build_fused(SUB, n_swdge=1):
    SLOTS = SUBS * SUB
    GROUPS = SLOTS // 128
    spg = SUB // 128
    ICOL = SUBS * (SUB // 16)

    nc = bacc.Bacc("TRN2", target_bir_lowering=False, debug=False,
                   dynamic_dma_scratch_size=131072, num_swdge_queues=n_swdge,
                   num_devices=CORES)
    xT = nc.dram_tensor("xT", [4, OWN], F32, kind="ExternalInput")
    wa1_t = nc.dram_tensor("wa1", [4, TW], F32, kind="ExternalInput")
    wa2_t = nc.dram_tensor("wa2", [17, TW], F32, kind="ExternalInput")
    wa3_t = nc.dram_tensor("wa3", [33, TW], F32, kind="ExternalInput")
    wm1_t = nc.dram_tensor("wm1", [3, 128], BF16, kind="ExternalInput")
    wm2_t = nc.dram_tensor("wm2", [16, 256], BF16, kind="ExternalInput")
    wm3_t = nc.dram_tensor("wm3", [32, 512], BF16, kind="ExternalInput")
    wuv_t = nc.dram_tensor("wuv", [65, 128], F32, kind="ExternalInput")
    wc_t = nc.dram_tensor("wc", [10, 64], BF16, kind="ExternalInput")
    w2_t = nc.dram_tensor("w2", [64, 16], BF16, kind="ExternalInput")
    b2_t = nc.dram_tensor("b2", [16, 1], F32, kind="ExternalInput")
    w3_t = nc.dram_tensor("w3", [16, 8], BF16, kind="ExternalInput")
    b3_t = nc.dram_tensor("bm3", [1, 1], F32, kind="ExternalInput")
    idx_t = nc.dram_tensor("idx", [TPC * 16, ICOL], I16, kind="ExternalInput")
    dloc_t = nc.dram_tensor("dloc", [TPC * 128, GROUPS], I16, kind="ExternalInput")
    attr_t = nc.dram_tensor("attr", [TPC * 128, GROUPS * 10], BF16,
                            kind="ExternalInput")
    out_t = nc.dram_tensor("out_slots", [TPC, SLOTS], BF16, kind="ExternalOutput")

    own = [nc.dram_tensor(f"own{l}", [OWN, TW], F32) for l in range(3)]
    gtbl = [nc.dram_tensor(f"gtbl{l}", [NP, TW], F32, addr_space="Shared")
            for l in range(3)]
    ownu = nc.dram_tensor("ownu", [OWN, TW], F32)
    ownv = nc.dram_tensor("ownv", [OWN, TW], F32)
    utbl = nc.dram_tensor("utbl", [NP, TW], F32, addr_space="Shared")
    vtbl = nc.dram_tensor("vtbl", [NP, TW], F32, addr_space="Shared")

    def allgather(src_t, dst_t):
        tc.strict_bb_all_engine_barrier()
        nc.gpsimd.collective_compute(
            "AllGather", mybir.AluOpType.bypass,
            replica_groups=[list(range(CORES))],
            ins=[src_t[:].opt()], outs=[dst_t[:].opt()])
        tc.strict_bb_all_engine_barrier()

    def load_idxs(psb, i):
        idxs = psb.tile([128, ICOL], I16)
        for k in range(8):
            nc.sync.dma_start(out=idxs[16 * k:16 * (k + 1), :],
                              in_=idx_t[bass.ts(i, 16), :])
        return idxs

    with tile.TileContext(nc) as tc:
        with tc.tile_pool(name="const", bufs=1) as cpool:
            wa1 = cpool.tile([4, TW], F32)
            nc.sync.dma_start(out=wa1[:], in_=wa1_t[:])
            wa2 = cpool.tile([17, TW], F32)
            nc.sync.dma_start(out=wa2[:], in_=wa2_t[:])
            wa3 = cpool.tile([33, TW], F32)
            nc.sync.dma_start(out=wa3[:], in_=wa3_t[:])
            wm1 = cpool.tile([3, 128], BF16)
            nc.sync.dma_start(out=wm1[:], in_=wm1_t[:])
            wm2 = cpool.tile([16, 256], BF16)
            nc.sync.dma_start(out=wm2[:], in_=wm2_t[:])
            wm3 = cpool.tile([32, 512], BF16)
            nc.sync.dma_start(out=wm3[:], in_=wm3_t[:])
            wuv = cpool.tile([65, 128], F32)
            nc.sync.dma_start(out=wuv[:], in_=wuv_t[:])
            wa2r = cpool.tile([1, TW], F32)
            nc.sync.dma_start(out=wa2r[:], in_=wa2_t[16:17, :])
            wa3r = cpool.tile([1, TW], F32)
            nc.sync.dma_start(out=wa3r[:], in_=wa3_t[32:33, :])
            wuvr = cpool.tile([1, 128], F32)
            nc.sync.dma_start(out=wuvr[:], in_=wuv_t[64:65, :])
            wc = cpool.tile([10, 64], BF16)
            nc.sync.dma_start(out=wc[:], in_=wc_t[:])
            w2 = cpool.tile([64, 16], BF16)
            nc.sync.dma_start(out=w2[:], in_=w2_t[:])
            b2s = cpool.tile([16, 1], F32)
            nc.sync.dma_start(out=b2s[:], in_=b2_t[:])
            w3 = cpool.tile([16, 8], BF16)
            nc.sync.dma_start(out=w3[:], in_=w3_t[:])
            b3s = cpool.tile([1, 1], F32)
            nc.sync.dma_start(out=b3s[:], in_=b3_t[:])
            iota = cpool.tile([128, 128], F32)
            nc.gpsimd.iota(iota[:], [[1, 128]], channel_multiplier=0,
                           allow_small_or_imprecise_dtypes=True)
            ident = cpool.tile([128, 128], F32)
            make_identity(nc, ident[:])
            identb = cpool.tile([128, 128], BF16)
            nc.vector.tensor_copy(out=identb[:], in_=ident[:])
            ones_r = cpool.tile([1, 128], F32)
            nc.vector.memset(ones_r[:], 1.0)

            pid = nc.sync.partition_id()

            # ------------- phase 0: layer-1 node table from own x shard
            with tc.tile_pool(name="n_in", bufs=2) as pin, \
                 tc.tile_pool(name="n_out", bufs=2) as pout, \
                 tc.tile_pool(name="n_ps", bufs=2, space="PSUM") as pps:
                with tc.For_i(0, OWN // NODE_CH, 1) as i:
                    pv = pin.tile([4, NODE_CH], F32)
                    nc.sync.dma_start(out=pv[:], in_=xT[:, bass.ts(i, NODE_CH)])
                    ob = pout.tile([128, NODE_CH // 128, TW], F32)
                    for k in range(NODE_CH // 128):
                        ps = pps.tile([128, TW], F32, space="PSUM")
                        nc.tensor.matmul(out=ps[:], lhsT=pv[:, k * 128:(k + 1) * 128],
                                         rhs=wa1[:], start=True, stop=True)
                        nc.scalar.copy(out=ob[:, k, :], in_=ps[:])
                    nc.sync.dma_start(
                        out=own[0][bass.ts(i, NODE_CH), :].rearrange(
                            "(k p) w -> p k w", p=128),
                        in_=ob[:])
            allgather(own[0], gtbl[0])

            # ------------- GAT edge phases
            layer_cfg = [
                (3, 128, 16, wm1, wa2, wa2r),
                (16, 256, 32, wm2, wa3, wa3r),
                (32, 512, 64, wm3, None, None),
            ]
            for l, (F_in, HF, F_out, wm, wa_next, wa_bias) in enumerate(layer_cfg):
                FH = HF // H
                gt_l = gtbl[l]
                with tc.tile_pool(name=f"eg{l}", bufs=2) as pg, \
                     tc.tile_pool(name=f"es{l}", bufs=2) as psb, \
                     tc.tile_pool(name=f"eps{l}", bufs=1, space="PSUM") as pps, \
                     tc.tile_pool(name=f"eac{l}", bufs=1, space="PSUM") as pac:
                    with tc.For_i(0, TPC, 1) as i:
                        idxs = load_idxs(psb, i)
                        dlc16 = psb.tile([128, GROUPS], I16)
                        nc.sync.dma_start(out=dlc16[:], in_=dloc_t[bass.ts(i, 128), :])
                        dlc = psb.tile([128, GROUPS], F32)
                        nc.vector.tensor_copy(out=dlc[:], in_=dlc16[:])
                        adn = psb.tile([128, 8], F32)
                        nc.sync.dma_start(
                            out=adn[:],
                            in_=gt_l[bass.ds((pid * TPC + i) * 128, 128),
                                     F_in + 8:F_in + 16])
                        adn_bf = psb.tile([128, 8], BF16)
                        nc.scalar.copy(out=adn_bf[:], in_=adn[:])

                        gt = pg.tile([128, GROUPS, TW], F32)
                        for s in range(SUBS):
                            nc.gpsimd.dma_gather(
                                out_ap=gt[:, s * spg:(s + 1) * spg, :],
                                in_ap=gt_l[s * CH:(s + 1) * CH, :],
                                idxs_ap=idxs[:, s * (SUB // 16):(s + 1) * (SUB // 16)],
                                num_idxs=SUB, num_idxs_reg=SUB,
                                elem_size=TW, single_packet=False,
                                queue_num=s % n_swdge)

                        vex = pg.tile([128, GROUPS, HF], BF16)
                        exb = psb.tile([128, GROUPS, H], BF16)
                        acc = pac.tile([128, HF], F32, space="PSUM")
                        den = pac.tile([128, H], F32, space="PSUM")
                        for g in range(GROUPS):
                            st = psb.tile([128, 128], BF16, tag="st")
                            nc.vector.tensor_scalar(
                                out=st[:], in0=iota[:], scalar1=dlc[:, g:g + 1],
                                scalar2=None, op0=mybir.AluOpType.is_equal)
                            tp = pps.tile([F_in, 128], F32, space="PSUM", tag="tp")
                            nc.tensor.transpose(out=tp[:], in_=gt[:, g, 0:F_in],
                                                identity=ident[:])
                            tpb = psb.tile([F_in, 128], BF16, tag="tpb")
                            nc.scalar.copy(out=tpb[:], in_=tp[:])
                            hp = pps.tile([128, HF], F32, space="PSUM", tag="hp")
                            nc.tensor.matmul(out=hp[:], lhsT=tpb[:], rhs=wm[:],
                                             start=True, stop=True)
                            sg = pps.tile([128, 128], BF16, space="PSUM", tag="sg")
                            nc.tensor.transpose(out=sg[:], in_=st[:],
                                                identity=identb[:])
                            sgb = psb.tile([128, 128], BF16, tag="sgb")
                            nc.scalar.copy(out=sgb[:], in_=sg[:])
                            ep = pps.tile([128, H], F32, space="PSUM", tag="ep")
                            nc.tensor.matmul(out=ep[:], lhsT=sgb[:], rhs=adn_bf[:],
                                             start=True, stop=True)
                            ef = psb.tile([128, H], F32, tag="ef")
                            nc.vector.tensor_add(out=ef[:], in0=ep[:],
                                                 in1=gt[:, g, F_in:F_in + 8])
                            eft = psb.tile([128, H], F32, tag="eft")
                            nc.vector.tensor_scalar(
                                out=eft[:], in0=ef[:], scalar1=0.2, scalar2=None,
                                op0=mybir.AluOpType.mult)
                            nc.vector.tensor_tensor(
                                out=ef[:], in0=ef[:], in1=eft[:],
                                op=mybir.AluOpType.max)
                            exf = psb.tile([128, H], F32, tag="exf")
                            nc.scalar.activation(out=exf[:], in_=ef[:],
                                                 func=mybir.ActivationFunctionType.Exp)
                            nc.vector.tensor_copy(out=exb[:, g, :], in_=exf[:])
                            for h in range(H):
                                if h % 2 == 0:
                                    nc.vector.tensor_scalar(
                                        out=vex[:, g, h * FH:(h + 1) * FH],
                                        in0=hp[:, h * FH:(h + 1) * FH],
                                        scalar1=exf[:, h:h + 1], scalar2=None,
                                        op0=mybir.AluOpType.mult)
                                else:
                                    nc.scalar.activation(
                                        out=vex[:, g, h * FH:(h + 1) * FH],
                                        in_=hp[:, h * FH:(h + 1) * FH],
                                        func=mybir.ActivationFunctionType.Copy,
                                        scale=exf[:, h:h + 1])
                            nc.tensor.matmul(out=acc[:], lhsT=st[:], rhs=vex[:, g, :],
                                             start=(g == 0), stop=(g == GROUPS - 1))
                            nc.tensor.matmul(out=den[:], lhsT=st[:], rhs=exb[:, g, :],
                                             start=(g == 0), stop=(g == GROUPS - 1))
                        # epilogue: out = mean_h acc_h / den_h  (raw, bias deferred)
                        dr = psb.tile([128, H], F32, tag="dr")
                        nc.vector.tensor_scalar(
                            out=dr[:], in0=den[:], scalar1=1e-30, scalar2=None,
                            op0=mybir.AluOpType.add)
                        nc.vector.reciprocal(out=dr[:], in_=dr[:])
                        nc.vector.tensor_scalar(out=dr[:], in0=dr[:], scalar1=1.0 / H,
                                                scalar2=None, op0=mybir.AluOpType.mult)
                        ot = psb.tile([128, H, F_out], F32, tag="ot")
                        for h in range(H):
                            if h % 2 == 0:
                                nc.vector.tensor_scalar(
                                    out=ot[:, h, :], in0=acc[:, h * FH:(h + 1) * FH],
                                    scalar1=dr[:, h:h + 1], scalar2=None,
                                    op0=mybir.AluOpType.mult)
                            else:
                                nc.scalar.activation(
                                    out=ot[:, h, :], in_=acc[:, h * FH:(h + 1) * FH],
                                    func=mybir.ActivationFunctionType.Copy,
                                    scale=dr[:, h:h + 1])
                        for step in [4, 2, 1]:
                            for h in range(step):
                                nc.vector.tensor_add(out=ot[:, h, :], in0=ot[:, h, :],
                                                     in1=ot[:, h + step, :])
                        # fused next-table epilogue
                        tpo = pps.tile([F_out, 128], F32, space="PSUM", tag="tpo")
                        nc.tensor.transpose(out=tpo[:], in_=ot[:, 0, :],
                                            identity=ident[:])
                        tpo_s = psb.tile([F_out, 128], F32, tag="tpos")
                        nc.scalar.copy(out=tpo_s[:], in_=tpo[:])
                        if l < 2:
                            nt = pps.tile([128, TW], F32, space="PSUM", tag="nt")
                            nc.tensor.matmul(out=nt[:], lhsT=tpo_s[:],
                                             rhs=wa_next[0:F_out, :],
                                             start=True, stop=False)
                            nc.tensor.matmul(out=nt[:], lhsT=ones_r[:],
                                             rhs=wa_bias[:],
                                             start=False, stop=True)
                            nts = psb.tile([128, TW], F32, tag="nts")
                            nc.scalar.copy(out=nts[:], in_=nt[:])
                            nc.sync.dma_start(out=own[l + 1][bass.ts(i, 128), :],
                                              in_=nts[:])
                        else:
                            nt = pps.tile([128, 128], F32, space="PSUM", tag="nt")
                            nc.tensor.matmul(out=nt[:], lhsT=tpo_s[:],
                                             rhs=wuv[0:64, :],
                                             start=True, stop=False)
                            nc.tensor.matmul(out=nt[:], lhsT=ones_r[:],
                                             rhs=wuvr[:],
                                             start=False, stop=True)
                            nts = psb.tile([128, 128], F32, tag="nts")
                            nc.scalar.copy(out=nts[:], in_=nt[:])
                            nc.sync.dma_start(out=ownu[bass.ts(i, 128), :],
                                              in_=nts[:, 0:64])
                            nc.sync.dma_start(out=ownv[bass.ts(i, 128), :],
                                              in_=nts[:, 64:128])
                if l < 2:
                    allgather(own[l + 1], gtbl[l + 1])
                else:
                    allgather(ownu, utbl)
                    allgather(ownv, vtbl)

            # ------------- phase 4: edge MLP
            with tc.tile_pool(name="mg", bufs=2) as pg, \
                 tc.tile_pool(name="ms", bufs=2) as psb, \
                 tc.tile_pool(name="mps", bufs=1, space="PSUM") as pps:
                with tc.For_i(0, TPC, 1) as i:
                    idxs = load_idxs(psb, i)
                    dlc16 = psb.tile([128, GROUPS], I16)
                    nc.sync.dma_start(out=dlc16[:], in_=dloc_t[bass.ts(i, 128), :])
                    dlc = psb.tile([128, GROUPS], F32)
                    nc.vector.tensor_copy(out=dlc[:], in_=dlc16[:])
                    vnd = psb.tile([128, 64], F32)
                    nc.sync.dma_start(
                        out=vnd[:],
                        in_=vtbl[bass.ds((pid * TPC + i) * 128, 128), :])
                    vnd_bf = psb.tile([128, 64], BF16)
                    nc.scalar.copy(out=vnd_bf[:], in_=vnd[:])
                    att = pg.tile([128, GROUPS, 10], BF16, tag="att")
                    nc.sync.dma_start(out=att[:].rearrange("p g w -> p (g w)"),
                                      in_=attr_t[bass.ts(i, 128), :])

                    gt = pg.tile([128, GROUPS, 64], F32)
                    for s in range(SUBS):
                        nc.gpsimd.dma_gather(
                            out_ap=gt[:, s * spg:(s + 1) * spg, :],
                            in_ap=utbl[s * CH:(s + 1) * CH, :],
                            idxs_ap=idxs[:, s * (SUB // 16):(s + 1) * (SUB // 16)],
                            num_idxs=SUB, num_idxs_reg=SUB,
                            elem_size=64, single_packet=False,
                            queue_num=s % n_swdge)

                    orow = psb.tile([1, GROUPS, 128], BF16, tag="orow")
                    for g in range(GROUPS):
                        st = psb.tile([128, 128], BF16, tag="st")
                        nc.vector.tensor_scalar(
                            out=st[:], in0=iota[:], scalar1=dlc[:, g:g + 1],
                            scalar2=None, op0=mybir.AluOpType.is_equal)
                        sg = pps.tile([128, 128], BF16, space="PSUM", tag="sg")
                        nc.tensor.transpose(out=sg[:], in_=st[:], identity=identb[:])
                        sgb = psb.tile([128, 128], BF16, tag="sgb")
                        nc.scalar.copy(out=sgb[:], in_=sg[:])
                        atp = pps.tile([10, 128], BF16, space="PSUM", tag="atp")
                        nc.tensor.transpose(out=atp[:], in_=att[:, g, :],
                                            identity=identb[:])
                        atpb = psb.tile([10, 128], BF16, tag="atpb")
                        nc.scalar.copy(out=atpb[:], in_=atp[:])
                        z1p = pps.tile([128, 64], F32, space="PSUM", tag="z1p")
                        nc.tensor.matmul(out=z1p[:], lhsT=atpb[:], rhs=wc[:],
                                         start=True, stop=False)
                        nc.tensor.matmul(out=z1p[:], lhsT=sgb[:], rhs=vnd_bf[:],
                                         start=False, stop=True)
                        z1 = psb.tile([128, 64], F32, tag="z1")
                        nc.vector.tensor_add(out=z1[:], in0=z1p[:], in1=gt[:, g, :])
                        z1s = psb.tile([128, 64], F32, tag="z1s")
                        nc.vector.tensor_scalar(
                            out=z1s[:], in0=z1[:], scalar1=0.12, scalar2=None,
                            op0=mybir.AluOpType.mult)
                        z1b = psb.tile([128, 64], BF16, tag="z1b")
                        nc.vector.tensor_tensor(
                            out=z1b[:], in0=z1[:], in1=z1s[:],
                            op=mybir.AluOpType.max)
                        z1t = pps.tile([64, 128], BF16, space="PSUM", tag="z1t")
                        nc.tensor.transpose(out=z1t[:], in_=z1b[:], identity=identb[:])
                        z1tb = psb.tile([64, 128], BF16, tag="z1tb")
                        nc.scalar.copy(out=z1tb[:], in_=z1t[:])
                        z2p = pps.tile([16, 128], F32, space="PSUM", tag="z2p")
                        nc.tensor.matmul(out=z2p[:], lhsT=w2[:], rhs=z1tb[:],
                                         start=True, stop=True)
                        z2f = psb.tile([16, 128], F32, tag="z2f")
                        nc.vector.tensor_scalar(
                            out=z2f[:], in0=z2p[:], scalar1=b2s[:, 0:1], scalar2=None,
                            op0=mybir.AluOpType.add)
                        z2s = psb.tile([16, 128], F32, tag="z2s")
                        nc.vector.tensor_scalar(
                            out=z2s[:], in0=z2f[:], scalar1=0.12, scalar2=None,
                            op0=mybir.AluOpType.mult)
                        z2b = psb.tile([16, 128], BF16, tag="z2b")
                        nc.vector.tensor_tensor(
                            out=z2b[:], in0=z2f[:], in1=z2s[:],
                            op=mybir.AluOpType.max)
                        z3p = pps.tile([8, 128], F32, space="PSUM", tag="z3p")
                        nc.tensor.matmul(out=z3p[:], lhsT=w3[:], rhs=z2b[:],
                                         start=True, stop=True)
                        nc.scalar.activation(out=orow[:, g, :], in_=z3p[0:1, :],
                                             func=mybir.ActivationFunctionType.Sigmoid,
                                             bias=b3s[:, 0:1])
                    nc.sync.dma_start(
                        out=out_t[bass.ts(i, 1), :],
                        in_=orow[:].rearrange("o g p -> o (g p)"))
    nc.compile()
    return nc


# ------------------------------------------------------------ runner
def _run_custom(nc, in_maps, n_cores):
    import jax
    from jax.sharding import Mesh, PartitionSpec, NamedSharding
    try:
        from jax.experimental.shard_map import shard_map
    except ImportError:
        from jax.shard_map import shard_map
    from concourse import bass2jax

    bass2jax.install_neuronx_cc_hook()
    partition_name = nc.partition_id_tensor.name if nc.partition_id_tensor else None
    in_names, out_names, out_avals = [], [], []
    for alloc in nc.m.functions[0].allocations:
        if not isinstance(alloc, mybir.MemoryLocationSet):
            continue
        name = alloc.memorylocations[0].name
        if alloc.kind == "ExternalInput":
            if name != partition_name:
                in_names.append(name)
        elif alloc.kind == "ExternalOutput":
            out_names.append(name)
            shape = tuple(alloc.tensor_shape)
            dtype = mybir.dt.np(alloc.dtype)
            out_avals.append(jax.core.ShapedArray(shape, dtype))
    n_params = len(in_names)
    n_outs = len(out_avals)
    all_in_names = list(in_names)
    if partition_name is not None:
        all_in_names.append(partition_name)

    def _body(*args):
        operands = list(args)
        if partition_name is not None:
            operands.append(bass2jax.partition_id_tensor())
        outs = bass2jax._bass_exec_p.bind(
            *operands,
            out_avals=tuple(out_avals),
            in_names=tuple(all_in_names),
            out_names=tuple(out_names),
            lowering_input_output_aliases=(),
            sim_require_finite=True,
            sim_require_nnan=True,
            nc=nc,
        )
        return tuple(outs)

    devices = jax.devices()[:n_cores]
    mesh = Mesh(np.asarray(devices), ("core",))
    sharding = NamedSharding(mesh, PartitionSpec("core"))
    in_specs = (PartitionSpec("core"),) * n_params
    out_specs = (PartitionSpec("core"),) * n_outs
    sharded = jax.jit(
        shard_map(_body, mesh=mesh, in_specs=in_specs, out_specs=out_specs,
                  check_rep=False),
        keep_unused=True)

    def gshape(a):
        return jax.ShapeDtypeStruct((n_cores * a.shape[0], *a.shape[1:]), a.dtype,
                                    sharding=sharding)
    sample = [np.asarray(in_maps[0][nm]) for nm in in_names]
    abstract = [gshape(a) for a in sample]

    box = {}
    def compile_job():
        try:
            box["c"] = sharded.lower(*abstract).compile()
        except Exception as exc:  # noqa: BLE001
            box["e"] = exc
    th = threading.Thread(target=compile_job)
    th.start()

    # build global host arrays while the compile thread runs; the compiled
    # call transfers them (explicit device_put hits a pathological one-time
    # init on the axon PJRT plugin, so pass numpy directly)
    np_args = [np.concatenate([np.asarray(m[nm]) for m in in_maps], axis=0)
               for nm in in_names]
    th.join()
    if "e" in box:
        raise box["e"]

    out_arrs = box["c"](*np_args)
    host = [np.asarray(a) for a in out_arrs]

    class _R:
        pass
    r = _R()
    r.results = [
        {nm: host[i].reshape(n_cores, *out_avals[i].shape)[c]
         for i, nm in enumerate(out_names)}
        for c in range(n_cores)
    ]
    return r


# ---------------------------------------------------------------- kernel
def _bf(x):
    return np.ascontiguousarray(x.astype(ml_dtypes.bfloat16))


def _waug_eff(W, a_s, a_d, b_prev):
    """Table row = [prev+b | al_s | al_d];  al = (prev+b) @ W_a."""
    Fin = W.shape[0]
    FHl = W.shape[1] // H
    Wal = np.einsum("ihf,hf->ih", W.reshape(Fin, H, FHl), a_s)
    Wad = np.einsum("ihf,hf->ih", W.reshape(Fin, H, FHl), a_d)
    wa = np.zeros((Fin + 1, TW), np.float32)
    wa[:Fin, :Fin] = np.eye(Fin, dtype=np.float32)
    wa[Fin, :Fin] = b_prev
    wa[:Fin, Fin:Fin + 8] = Wal
    wa[Fin, Fin:Fin + 8] = b_prev @ Wal
    wa[:Fin, Fin + 8:Fin + 16] = Wad
    wa[Fin, Fin + 8:Fin + 16] = b_prev @ Wad
    return wa


def kernel(**inputs):
    # warm the jax/axon backend while we build and prep
    def _warm():
        try:
            import jax
            jax.devices()
        except Exception:  # noqa: BLE001
            pass
    warm_th = threading.Thread(target=_warm)
    warm_th.start()

    x = np.asarray(inputs["x"], np.float32)
    ei = np.asarray(inputs["edge_index"])
    ea = np.asarray(inputs["edge_attr"], np.float32)
    cores = list(range(CORES))

    src = ei[0].astype(np.int64)
    dst = ei[1].astype(np.int64)
    loop = np.arange(N, dtype=np.int64)
    src_sl = np.concatenate([src, loop])
    dst_sl = np.concatenate([dst, loop])

    # dynamic SUB from actual chunk occupancy
    key = (dst_sl >> 7) * SUBS + src_sl // CH
    counts = np.bincount(key, minlength=TILES * SUBS)
    SUB = int(-(-counts.max() // 128) * 128)
    SLOTS = SUBS * SUB
    GROUPS = SLOTS // 128

    # build + trace + compile in a worker thread; host prep + transfers here
    build_box = {}
    def build_job():
        try:
            build_box["nc"] = build_fused(SUB)
        except Exception as exc:  # noqa: BLE001
            build_box["e"] = exc
    build_th = threading.Thread(target=build_job)
    build_th.start()

    idx_w, dl, slot_of = _sort_edges(src_sl, dst_sl, SUB)

    idx_sh = [np.ascontiguousarray(
        idx_w[cc * TPC:(cc + 1) * TPC].reshape(TPC * 16, -1)) for cc in cores]
    dl_sh = [np.ascontiguousarray(
        dl[cc * TPC:(cc + 1) * TPC].reshape(TPC * 128, -1)) for cc in cores]

    # attr in slot space, 10 cols, bf16
    attr_slot = np.zeros((TILES * SLOTS, 10), ml_dtypes.bfloat16)
    attr_slot[slot_of[:E]] = ea.astype(ml_dtypes.bfloat16)
    attr_slot = attr_slot.reshape(TILES, GROUPS, 128, 10).transpose(0, 2, 1, 3)
    attr_sh = [np.ascontiguousarray(
        attr_slot[cc * TPC:(cc + 1) * TPC].reshape(TPC * 128, GROUPS * 10))
        for cc in cores]

    # x shards: [4, OWN] per core (row 3 = ones)
    xT = np.zeros((4, NP), np.float32)
    xT[:3, :N] = x.T
    xT[3, :] = 1.0
    xT_sh = [np.ascontiguousarray(xT[:, cc * OWN:(cc + 1) * OWN]) for cc in cores]

    W1 = np.asarray(inputs["W1"], np.float32)
    W2g = np.asarray(inputs["W2"], np.float32)
    W3g = np.asarray(inputs["W3"], np.float32)
    b1 = np.asarray(inputs["b1"], np.float32)
    b2g = np.asarray(inputs["b2"], np.float32)
    b3 = np.asarray(inputs["b3"], np.float32)
    wa1 = _waug_eff(W1, np.asarray(inputs["as1"], np.float32),
                    np.asarray(inputs["ad1"], np.float32), np.zeros(3, np.float32))
    wa2 = _waug_eff(W2g, np.asarray(inputs["as2"], np.float32),
                    np.asarray(inputs["ad2"], np.float32), b1)
    wa3 = _waug_eff(W3g, np.asarray(inputs["as3"], np.float32),
                    np.asarray(inputs["ad3"], np.float32), b2g)

    Wm1 = np.asarray(inputs["Wm1"], np.float32)
    bm1 = np.asarray(inputs["bm1"], np.float32)
    Wm2 = np.asarray(inputs["Wm2"], np.float32)
    bm2 = np.asarray(inputs["bm2"], np.float32)
    Wm3 = np.asarray(inputs["Wm3"], np.float32)
    bm3 = np.asarray(inputs["bm3"], np.float32)
    Wu, Wv, Wc_ = Wm1[:64], Wm1[64:128], Wm1[128:138]
    wuv = np.zeros((65, 128), np.float32)
    wuv[:64, :64] = Wu
    wuv[64, :64] = b3 @ Wu + 0.5 * bm1
    wuv[:64, 64:] = Wv
    wuv[64, 64:] = b3 @ Wv + 0.5 * bm1
    w3p = np.zeros((16, 8), np.float32)
    w3p[:, 0:1] = Wm3

    in_maps = []
    for cc in cores:
        in_maps.append({
            "xT": xT_sh[cc], "wa1": wa1, "wa2": wa2, "wa3": wa3,
            "wm1": _bf(W1), "wm2": _bf(W2g), "wm3": _bf(W3g),
            "wuv": wuv, "wc": _bf(Wc_), "w2": _bf(Wm2),
            "b2": bm2.reshape(16, 1), "w3": _bf(w3p), "bm3": bm3.reshape(1, 1),
            "idx": idx_sh[cc], "dloc": dl_sh[cc], "attr": attr_sh[cc],
        })

    build_th.join()
    if "e" in build_box:
        raise build_box["e"]
    nc = build_box["nc"]
    warm_th.join()

    try:
        res = _run_custom(nc, in_maps, CORES)
    except Exception:  # noqa: BLE001
        res = run_bass_kernel_spmd(nc, in_maps, cores)
    oslots = np.concatenate([r["out_slots"] for r in res.results], 0)
    oslots = oslots.reshape(-1).astype(np.float32)
    out = oslots[slot_of[:E]]
    return out.reshape(E, 1)


# revision 11
# speedup vs baseline: 2.9053x; 2.9053x over previous
"""Trainium2 Bass kernel for nn_BasicAttentionModel (3-layer GAT + edge MLP).

Fused single-launch design (8-core SPMD, dst-partitioned edges):
  - One Bass program holds all four phases (GAT x3 + edge MLP); intermediate
    node tables never leave the device.  Each core owns 98 consecutive
    128-node tiles (its dst range) and processes only its own edges; the
    per-layer node tables [prev+b | al_s | al_d] are computed in the edge
    phase epilogue (transpose + matmul with an augmented weight) and
    AllGathered across the 8 cores between phases.
  - Edges (with self-loops) are dst-sorted into tiles and grouped into 4
    src-chunk sub-tiles (int16 gather indices); the sub-tile size SUB is
    sized from the actual max chunk occupancy, rounded up to 128.
  - The edge MLP reuses the same slot layout (self-loop slots discarded on
    the host); edge_attr ships as 10-col bf16 in slot order; gather indices
    ship once (16 rows per tile) and are replicated across partitions on
    device.
  - The runner overlaps program build + jit trace + neuronx compile (worker
    thread) with host-side edge sorting and per-device input streaming
    (main thread).
"""
import threading
import numpy as np
import ml_dtypes

import concourse.bacc as bacc
import concourse.bass as bass
import concourse.mybir as mybir
import concourse.tile as tile
from concourse.bass_utils import run_bass_kernel_spmd
from concourse.masks import make_identity

F32 = mybir.dt.float32
BF16 = mybir.dt.bfloat16
I16 = mybir.dt.int16

N = 100000
E = 1600000
H = 8
CORES = 8
NP = 100352          # 784 * 128, divisible by 4 chunks of 25088
CH = 25088
TILE_N = 128
SUBS = 4
TILES = NP // TILE_N          # 784
TPC = TILES // CORES          # 98
OWN = TPC * TILE_N            # 12544 nodes per core
NODE_CH = 1792                # node-phase trip (OWN = 7 * 1792)
TW = 64                       # table row width (floats) = 256B


# ------------------------------------------------------------ host prep
def _sort_edges(src, dst, SUB):
    """Vectorized dst-tile / src-chunk slotting.
    Returns idx (int16, [TILES,16,SUBS*SUB/16]), dloc (f32
    [TILES,128,GROUPS]), slot_of (edge -> global slot)."""
    SLOTS = SUBS * SUB
    GROUPS = SLOTS // 128
    tile_i = (dst >> 7).astype(np.int64)
    chunk = src // CH
    key = tile_i * SUBS + chunk
    order = np.argsort(key, kind="stable")
    ks = key[order]
    starts = np.searchsorted(ks, np.arange(TILES * SUBS))
    rank = np.arange(len(ks)) - starts[ks]
    slot_sorted = tile_i[order] * SLOTS + chunk[order] * SUB + rank
    slot_of = np.empty(len(ks), np.int64)
    slot_of[order] = slot_sorted

    idx_flat = np.zeros(TILES * SLOTS, np.int16)
    idx_flat[slot_sorted] = (src[order] - chunk[order] * CH).astype(np.int16)
    dloc_flat = np.full(TILES * SLOTS, 255, np.int16)
    dloc_flat[slot_sorted] = (dst[order] - tile_i[order] * TILE_N).astype(np.int16)

    # wrap idx for dma_gather: j -> partition j%16, col j//16 (16 rows/tile;
    # replication to 128 partitions happens on device)
    w = idx_flat.reshape(TILES, SUBS, SUB // 16, 16)
    idx_w = np.transpose(w, (0, 3, 1, 2)).reshape(TILES, 16, SUBS * (SUB // 16))
    # dloc arranged [TILES, 128, GROUPS]: slot = g*128+p
    dl = dloc_flat.reshape(TILES, GROUPS, 128).transpose(0, 2, 1).copy()
    return idx_w, dl, slot_of


# ------------------------------------------------------------ program
def build_fused(SUB, n_swdge=1):
    SLOTS = SUBS * SUB
    GROUPS = SLOTS // 128
    spg = SUB // 128
    ICOL = SUBS * (SUB // 16)

    nc = bacc.Bacc("TRN2", target_bir_lowering=False, debug=False,
                   dynamic_dma_scratch_size=131072, num_swdge_queues=n_swdge,
                   num_devices=CORES)
    xT = nc.dram_tensor("xT", [4, OWN], F32, kind="ExternalInput")
    wa1_t = nc.dram_tensor("wa1", [4, TW], F32, kind="ExternalInput")
    wa2_t = nc.dram_tensor("wa2", [17, TW], F32, kind="ExternalInput")
    wa3_t = nc.dram_tensor("wa3", [33, TW], F32, kind="ExternalInput")
    wm1_t = nc.dram_tensor("wm1", [3, 128], BF16, kind="ExternalInput")
    wm2_t = nc.dram_tensor("wm2", [16, 256], BF16, kind="ExternalInput")
    wm3_t = nc.dram_tensor("wm3", [32, 512], BF16, kind="ExternalInput")
    wuv_t = nc.dram_tensor("wuv", [65, 128], F32, kind="ExternalInput")
    wc_t = nc.dram_tensor("wc", [10, 64], BF16, kind="ExternalInput")
    w2_t = nc.dram_tensor("w2", [64, 16], BF16, kind="ExternalInput")
    b2_t = nc.dram_tensor("b2", [16, 1], F32, kind="ExternalInput")
    w3_t = nc.dram_tensor("w3", [16, 8], BF16, kind="ExternalInput")
    b3_t = nc.dram_tensor("bm3", [1, 1], F32, kind="ExternalInput")
    idx_t = nc.dram_tensor("idx", [TPC * 16, ICOL], I16, kind="ExternalInput")
    dloc_t = nc.dram_tensor("dloc", [TPC * 128, GROUPS], I16, kind="ExternalInput")
    attr_t = nc.dram_tensor("attr", [TPC * 128, GROUPS * 10], BF16,
                            kind="ExternalInput")
    out_t = nc.dram_tensor("out_slots", [TPC, SLOTS], BF16, kind="ExternalOutput")

    own = [nc.dram_tensor(f"own{l}", [OWN, TW], F32) for l in range(3)]
    gtbl = [nc.dram_tensor(f"gtbl{l}", [NP, TW], F32, addr_space="Shared")
            for l in range(3)]
    ownu = nc.dram_tensor("ownu", [OWN, TW], F32)
    ownv = nc.dram_tensor("ownv", [OWN, TW], F32)
    utbl = nc.dram_tensor("utbl", [NP, TW], F32, addr_space="Shared")
    vtbl = nc.dram_tensor("vtbl", [NP, TW], F32, addr_space="Shared")

    def allgather(src_t, dst_t):
        tc.strict_bb_all_engine_barrier()
        nc.gpsimd.collective_compute(
            "AllGather", mybir.AluOpType.bypass,
            replica_groups=[list(range(CORES))],
            ins=[src_t[:].opt()], outs=[dst_t[:].opt()])
        tc.strict_bb_all_engine_barrier()

    def load_idxs(psb, i):
        idxs = psb.tile([128, ICOL], I16)
        for k in range(8):
            nc.sync.dma_start(out=idxs[16 * k:16 * (k + 1), :],
                              in_=idx_t[bass.ts(i, 16), :])
        return idxs

    with tile.TileContext(nc) as tc:
        with tc.tile_pool(name="const", bufs=1) as cpool:
            wa1 = cpool.tile([4, TW], F32)
            nc.sync.dma_start(out=wa1[:], in_=wa1_t[:])
            wa2 = cpool.tile([17, TW], F32)
            nc.sync.dma_start(out=wa2[:], in_=wa2_t[:])
            wa3 = cpool.tile([33, TW], F32)
            nc.sync.dma_start(out=wa3[:], in_=wa3_t[:])
            wm1 = cpool.tile([3, 128], BF16)
            nc.sync.dma_start(out=wm1[:], in_=wm1_t[:])
            wm2 = cpool.tile([16, 256], BF16)
            nc.sync.dma_start(out=wm2[:], in_=wm2_t[:])
            wm3 = cpool.tile([32, 512], BF16)
            nc.sync.dma_start(out=wm3[:], in_=wm3_t[:])
            wuv = cpool.tile([65, 128], F32)
            nc.sync.dma_start(out=wuv[:], in_=wuv_t[:])
            wa2r = cpool.tile([1, TW], F32)
            nc.sync.dma_start(out=wa2r[:], in_=wa2_t[16:17, :])
            wa3r = cpool.tile([1, TW], F32)
            nc.sync.dma_start(out=wa3r[:], in_=wa3_t[32:33, :])
            wuvr = cpool.tile([1, 128], F32)
            nc.sync.dma_start(out=wuvr[:], in_=wuv_t[64:65, :])
            wc = cpool.tile([10, 64], BF16)
            nc.sync.dma_start(out=wc[:], in_=wc_t[:])
            w2 = cpool.tile([64, 16], BF16)
            nc.sync.dma_start(out=w2[:], in_=w2_t[:])
            b2s = cpool.tile([16, 1], F32)
            nc.sync.dma_start(out=b2s[:], in_=b2_t[:])
            w3 = cpool.tile([16, 8], BF16)
            nc.sync.dma_start(out=w3[:], in_=w3_t[:])
            b3s = cpool.tile([1, 1], F32)
            nc.sync.dma_start(out=b3s[:], in_=b3_t[:])
            iota = cpool.tile([128, 128], F32)
            nc.gpsimd.iota(iota[:], [[1, 128]], channel_multiplier=0,
                           allow_small_or_imprecise_dtypes=True)
            ident = cpool.tile([128, 128], F32)
            make_identity(nc, ident[:])
            identb = cpool.tile([128, 128], BF16)
            nc.vector.tensor_copy(out=identb[:], in_=ident[:])
            ones_r = cpool.tile([1, 128], F32)
            nc.vector.memset(ones_r[:], 1.0)

            pid = nc.sync.partition_id()

            # ------------- phase 0: layer-1 node table from own x shard
            with tc.tile_pool(name="n_in", bufs=2) as pin, \
                 tc.tile_pool(name="n_out", bufs=2) as pout, \
                 tc.tile_pool(name="n_ps", bufs=2, space="PSUM") as pps:
                with tc.For_i(0, OWN // NODE_CH, 1) as i:
                    pv = pin.tile([4, NODE_CH], F32)
                    nc.sync.dma_start(out=pv[:], in_=xT[:, bass.ts(i, NODE_CH)])
                    ob = pout.tile([128, NODE_CH // 128, TW], F32)
                    for k in range(NODE_CH // 128):
                        ps = pps.tile([128, TW], F32, space="PSUM")
                        nc.tensor.matmul(out=ps[:], lhsT=pv[:, k * 128:(k + 1) * 128],
                                         rhs=wa1[:], start=True, stop=True)
                        nc.scalar.copy(out=ob[:, k, :], in_=ps[:])
                    nc.sync.dma_start(
                        out=own[0][bass.ts(i, NODE_CH), :].rearrange(
                            "(k p) w -> p k w", p=128),
                        in_=ob[:])
            allgather(own[0], gtbl[0])

            # ------------- GAT edge phases
            layer_cfg = [
                (3, 128, 16, wm1, wa2, wa2r),
                (16, 256, 32, wm2, wa3, wa3r),
                (32, 512, 64, wm3, None, None),
            ]
            for l, (F_in, HF, F_out, wm, wa_next, wa_bias) in enumerate(layer_cfg):
                FH = HF // H
                gt_l = gtbl[l]
                with tc.tile_pool(name=f"eg{l}", bufs=2) as pg, \
                     tc.tile_pool(name=f"es{l}", bufs=2) as psb, \
                     tc.tile_pool(name=f"eps{l}", bufs=1, space="PSUM") as pps, \
                     tc.tile_pool(name=f"eac{l}", bufs=1, space="PSUM") as pac:
                    with tc.For_i(0, TPC, 1) as i:
                        idxs = load_idxs(psb, i)
                        dlc16 = psb.tile([128, GROUPS], I16)
                        nc.sync.dma_start(out=dlc16[:], in_=dloc_t[bass.ts(i, 128), :])
                        dlc = psb.tile([128, GROUPS], F32)
                        nc.vector.tensor_copy(out=dlc[:], in_=dlc16[:])
                        adn = psb.tile([128, 8], F32)
                        nc.sync.dma_start(
                            out=adn[:],
                            in_=gt_l[bass.ds((pid * TPC + i) * 128, 128),
                                     F_in + 8:F_in + 16])
                        adn_bf = psb.tile([128, 8], BF16)
                        nc.scalar.copy(out=adn_bf[:], in_=adn[:])

                        gt = pg.tile([128, GROUPS, TW], F32)
                        for s in range(SUBS):
                            nc.gpsimd.dma_gather(
                                out_ap=gt[:, s * spg:(s + 1) * spg, :],
                                in_ap=gt_l[s * CH:(s + 1) * CH, :],
                                idxs_ap=idxs[:, s * (SUB // 16):(s + 1) * (SUB // 16)],
                                num_idxs=SUB, num_idxs_reg=SUB,
                                elem_size=TW, single_packet=False,
                                queue_num=s % n_swdge)

                        vex = pg.tile([128, GROUPS, HF], BF16)
                        exb = psb.tile([128, GROUPS, H], BF16)
                        acc = pac.tile([128, HF], F32, space="PSUM")
                        den = pac.tile([128, H], F32, space="PSUM")
                        for g in range(GROUPS):
                            st = psb.tile([128, 128], BF16, tag="st")
                            nc.vector.tensor_scalar(
                                out=st[:], in0=iota[:], scalar1=dlc[:, g:g + 1],
                                scalar2=None, op0=mybir.AluOpType.is_equal)
                            tp = pps.tile([F_in, 128], F32, space="PSUM", tag="tp")
                            nc.tensor.transpose(out=tp[:], in_=gt[:, g, 0:F_in],
                                                identity=ident[:])
                            tpb = psb.tile([F_in, 128], BF16, tag="tpb")
                            nc.scalar.copy(out=tpb[:], in_=tp[:])
                            hp = pps.tile([128, HF], F32, space="PSUM", tag="hp")
                            nc.tensor.matmul(out=hp[:], lhsT=tpb[:], rhs=wm[:],
                                             start=True, stop=True)
                            sg = pps.tile([128, 128], BF16, space="PSUM", tag="sg")
                            nc.tensor.transpose(out=sg[:], in_=st[:],
                                                identity=identb[:])
                            sgb = psb.tile([128, 128], BF16, tag="sgb")
                            nc.scalar.copy(out=sgb[:], in_=sg[:])
                            ep = pps.tile([128, H], F32, space="PSUM", tag="ep")
                            nc.tensor.matmul(out=ep[:], lhsT=sgb[:], rhs=adn_bf[:],
                                             start=True, stop=True)
                            ef = psb.tile([128, H], F32, tag="ef")
                            nc.vector.tensor_add(out=ef[:], in0=ep[:],
                                                 in1=gt[:, g, F_in:F_in + 8])
                            eft = psb.tile([128, H], F32, tag="eft")
                            nc.vector.tensor_scalar(
                                out=eft[:], in0=ef[:], scalar1=0.2, scalar2=None,
                                op0=mybir.AluOpType.mult)
                            nc.vector.tensor_tensor(
                                out=ef[:], in0=ef[:], in1=eft[:],
                                op=mybir.AluOpType.max)
                            exf = psb.tile([128, H], F32, tag="exf")
                            nc.scalar.activation(out=exf[:], in_=ef[:],
                                                 func=mybir.ActivationFunctionType.Exp)
                            nc.vector.tensor_copy(out=exb[:, g, :], in_=exf[:])
                            for h in range(H):
                                if h % 2 == 0:
                                    nc.vector.tensor_scalar(
                                        out=vex[:, g, h * FH:(h + 1) * FH],
                                        in0=hp[:, h * FH:(h + 1) * FH],
                                        scalar1=exf[:, h:h + 1], scalar2=None,
                                        op0=mybir.AluOpType.mult)
                                else:
                                    nc.scalar.activation(
                                        out=vex[:, g, h * FH:(h + 1) * FH],
                                        in_=hp[:, h * FH:(h + 1) * FH],
                                        func=mybir.ActivationFunctionType.Copy,
                                        scale=exf[:, h:h + 1])
                            nc.tensor.matmul(out=acc[:], lhsT=st[:], rhs=vex[:, g, :],
                                             start=(g == 0), stop=(g == GROUPS - 1))
                            nc.tensor.matmul(out=den[:], lhsT=st[:], rhs=exb[:, g, :],
                                             start=(g == 0), stop=(g == GROUPS - 1))
                        # epilogue: out = sum_h acc_h / den_h (1/H folded into
                        # the next-layer weights on host; bias deferred)
                        dr = psb.tile([128, H], F32, tag="dr")
                        nc.vector.tensor_scalar(
                            out=dr[:], in0=den[:], scalar1=1e-30, scalar2=None,
                            op0=mybir.AluOpType.add)
                        nc.vector.reciprocal(out=dr[:], in_=dr[:])
                        ot = psb.tile([128, H, F_out], F32, tag="ot")
                        for h in range(H):
                            if h % 2 == 0:
                                nc.vector.tensor_scalar(
                                    out=ot[:, h, :], in0=acc[:, h * FH:(h + 1) * FH],
                                    scalar1=dr[:, h:h + 1], scalar2=None,
                                    op0=mybir.AluOpType.mult)
                            else:
                                nc.scalar.activation(
                                    out=ot[:, h, :], in_=acc[:, h * FH:(h + 1) * FH],
                                    func=mybir.ActivationFunctionType.Copy,
                                    scale=dr[:, h:h + 1])
                        for step in [4, 2, 1]:
                            for h in range(step):
                                nc.vector.tensor_add(out=ot[:, h, :], in0=ot[:, h, :],
                                                     in1=ot[:, h + step, :])
                        # fused next-table epilogue
                        tpo = pps.tile([F_out, 128], F32, space="PSUM", tag="tpo")
                        nc.tensor.transpose(out=tpo[:], in_=ot[:, 0, :],
                                            identity=ident[:])
                        tpo_s = psb.tile([F_out, 128], F32, tag="tpos")
                        nc.scalar.copy(out=tpo_s[:], in_=tpo[:])
                        if l < 2:
                            nt = pps.tile([128, TW], F32, space="PSUM", tag="nt")
                            nc.tensor.matmul(out=nt[:], lhsT=tpo_s[:],
                                             rhs=wa_next[0:F_out, :],
                                             start=True, stop=False)
                            nc.tensor.matmul(out=nt[:], lhsT=ones_r[:],
                                             rhs=wa_bias[:],
                                             start=False, stop=True)
                            nts = psb.tile([128, TW], F32, tag="nts")
                            nc.scalar.copy(out=nts[:], in_=nt[:])
                            nc.sync.dma_start(out=own[l + 1][bass.ts(i, 128), :],
                                              in_=nts[:])
                        else:
                            nt = pps.tile([128, 128], F32, space="PSUM", tag="nt")
                            nc.tensor.matmul(out=nt[:], lhsT=tpo_s[:],
                                             rhs=wuv[0:64, :],
                                             start=True, stop=False)
                            nc.tensor.matmul(out=nt[:], lhsT=ones_r[:],
                                             rhs=wuvr[:],
                                             start=False, stop=True)
                            nts = psb.tile([128, 128], F32, tag="nts")
                            nc.scalar.copy(out=nts[:], in_=nt[:])
                            nc.sync.dma_start(out=ownu[bass.ts(i, 128), :],
                                              in_=nts[:, 0:64])
                            nc.sync.dma_start(out=ownv[bass.ts(i, 128), :],
                                              in_=nts[:, 64:128])
                if l < 2:
                    allgather(own[l + 1], gtbl[l + 1])
                else:
                    allgather(ownu, utbl)
                    allgather(ownv, vtbl)

            # ------------- phase 4: edge MLP
            with tc.tile_pool(name="mg", bufs=2) as pg, \
                 tc.tile_pool(name="ms", bufs=2) as psb, \
                 tc.tile_pool(name="mps", bufs=1, space="PSUM") as pps:
                with tc.For_i(0, TPC, 1) as i:
                    idxs = load_idxs(psb, i)
                    dlc16 = psb.tile([128, GROUPS], I16)
                    nc.sync.dma_start(out=dlc16[:], in_=dloc_t[bass.ts(i, 128), :])
                    dlc = psb.tile([128, GROUPS], F32)
                    nc.vector.tensor_copy(out=dlc[:], in_=dlc16[:])
                    vnd = psb.tile([128, 64], F32)
                    nc.sync.dma_start(
                        out=vnd[:],
                        in_=vtbl[bass.ds((pid * TPC + i) * 128, 128), :])
                    vnd_bf = psb.tile([128, 64], BF16)
                    nc.scalar.copy(out=vnd_bf[:], in_=vnd[:])
                    att = pg.tile([128, GROUPS, 10], BF16, tag="att")
                    nc.sync.dma_start(out=att[:].rearrange("p g w -> p (g w)"),
                                      in_=attr_t[bass.ts(i, 128), :])

                    gt = pg.tile([128, GROUPS, 64], F32)
                    for s in range(SUBS):
                        nc.gpsimd.dma_gather(
                            out_ap=gt[:, s * spg:(s + 1) * spg, :],
                            in_ap=utbl[s * CH:(s + 1) * CH, :],
                            idxs_ap=idxs[:, s * (SUB // 16):(s + 1) * (SUB // 16)],
                            num_idxs=SUB, num_idxs_reg=SUB,
                            elem_size=64, single_packet=False,
                            queue_num=s % n_swdge)

                    orow = psb.tile([1, GROUPS, 128], BF16, tag="orow")
                    for g in range(GROUPS):
                        st = psb.tile([128, 128], BF16, tag="st")
                        nc.vector.tensor_scalar(
                            out=st[:], in0=iota[:], scalar1=dlc[:, g:g + 1],
                            scalar2=None, op0=mybir.AluOpType.is_equal)
                        sg = pps.tile([128, 128], BF16, space="PSUM", tag="sg")
                        nc.tensor.transpose(out=sg[:], in_=st[:], identity=identb[:])
                        sgb = psb.tile([128, 128], BF16, tag="sgb")
                        nc.scalar.copy(out=sgb[:], in_=sg[:])
                        atp = pps.tile([10, 128], BF16, space="PSUM", tag="atp")
                        nc.tensor.transpose(out=atp[:], in_=att[:, g, :],
                                            identity=identb[:])
                        atpb = psb.tile([10, 128], BF16, tag="atpb")
                        nc.scalar.copy(out=atpb[:], in_=atp[:])
                        z1p = pps.tile([128, 64], F32, space="PSUM", tag="z1p")
                        nc.tensor.matmul(out=z1p[:], lhsT=atpb[:], rhs=wc[:],
                                         start=True, stop=False)
                        nc.tensor.matmul(out=z1p[:], lhsT=sgb[:], rhs=vnd_bf[:],
                                         start=False, stop=True)
                        z1 = psb.tile([128, 64], F32, tag="z1")
                        nc.vector.tensor_add(out=z1[:], in0=z1p[:], in1=gt[:, g, :])
                        z1s = psb.tile([128, 64], F32, tag="z1s")
                        nc.vector.tensor_scalar(
                            out=z1s[:], in0=z1[:], scalar1=0.12, scalar2=None,
                            op0=mybir.AluOpType.mult)
                        z1b = psb.tile([128, 64], BF16, tag="z1b")
                        nc.vector.tensor_tensor(
                            out=z1b[:], in0=z1[:], in1=z1s[:],
                            op=mybir.AluOpType.max)
                        z1t = pps.tile([64, 128], BF16, space="PSUM", tag="z1t")
                        nc.tensor.transpose(out=z1t[:], in_=z1b[:], identity=identb[:])
                        z1tb = psb.tile([64, 128], BF16, tag="z1tb")
                        nc.scalar.copy(out=z1tb[:], in_=z1t[:])
                        z2p = pps.tile([16, 128], F32, space="PSUM", tag="z2p")
                        nc.tensor.matmul(out=z2p[:], lhsT=w2[:], rhs=z1tb[:],
                                         start=True, stop=True)
                        z2f = psb.tile([16, 128], F32, tag="z2f")
                        nc.vector.tensor_scalar(
                            out=z2f[:], in0=z2p[:], scalar1=b2s[:, 0:1], scalar2=None,
                            op0=mybir.AluOpType.add)
                        z2s = psb.tile([16, 128], F32, tag="z2s")
                        nc.vector.tensor_scalar(
                            out=z2s[:], in0=z2f[:], scalar1=0.12, scalar2=None,
                            op0=mybir.AluOpType.mult)
                        z2b = psb.tile([16, 128], BF16, tag="z2b")
                        nc.vector.tensor_tensor(
                            out=z2b[:], in0=z2f[:], in1=z2s[:],
                            op=mybir.AluOpType.max)
                        z3p = pps.tile([8, 128], F32, space="PSUM", tag="z3p")
                        nc.tensor.matmul(out=z3p[:], lhsT=w3[:], rhs=z2b[:],
                                         start=True, stop=True)
                        nc.scalar.activation(out=orow[:, g, :], in_=z3p[0:1, :],
                                             func=mybir.ActivationFunctionType.Sigmoid,
                                             bias=b3s[:, 0:1])
                    nc.sync.dma_start(
                        out=out_t[bass.ts(i, 1), :],
                        in_=orow[:].rearrange("o g p -> o (g p)"))
    nc.compile()
    return nc


# ------------------------------------------------------------ runner
def _run_custom(nc, in_maps, n_cores, global_arrays=None):
    import jax
    from jax.sharding import Mesh, PartitionSpec, NamedSharding
    try:
        from jax.experimental.shard_map import shard_map
    except ImportError:
        from jax.shard_map import shard_map
    from concourse import bass2jax

    bass2jax.install_neuronx_cc_hook()
    partition_name = nc.partition_id_tensor.name if nc.partition_id_tensor else None
    in_names, out_names, out_avals = [], [], []
    for alloc in nc.m.functions[0].allocations:
        if not isinstance(alloc, mybir.MemoryLocationSet):
            continue
        name = alloc.memorylocations[0].name
        if alloc.kind == "ExternalInput":
            if name != partition_name:
                in_names.append(name)
        elif alloc.kind == "ExternalOutput":
            out_names.append(name)
            shape = tuple(alloc.tensor_shape)
            dtype = mybir.dt.np(alloc.dtype)
            out_avals.append(jax.core.ShapedArray(shape, dtype))
    n_params = len(in_names)
    n_outs = len(out_avals)
    all_in_names = list(in_names)
    if partition_name is not None:
        all_in_names.append(partition_name)

    def _body(*args):
        operands = list(args)
        if partition_name is not None:
            operands.append(bass2jax.partition_id_tensor())
        outs = bass2jax._bass_exec_p.bind(
            *operands,
            out_avals=tuple(out_avals),
            in_names=tuple(all_in_names),
            out_names=tuple(out_names),
            lowering_input_output_aliases=(),
            sim_require_finite=True,
            sim_require_nnan=True,
            nc=nc,
        )
        return tuple(outs)

    devices = jax.devices()[:n_cores]
    mesh = Mesh(np.asarray(devices), ("core",))
    sharding = NamedSharding(mesh, PartitionSpec("core"))
    in_specs = (PartitionSpec("core"),) * n_params
    out_specs = (PartitionSpec("core"),) * n_outs
    sharded = jax.jit(
        shard_map(_body, mesh=mesh, in_specs=in_specs, out_specs=out_specs,
                  check_rep=False),
        keep_unused=True)

    def gshape(a):
        return jax.ShapeDtypeStruct((n_cores * a.shape[0], *a.shape[1:]), a.dtype,
                                    sharding=sharding)
    sample = [np.asarray(in_maps[0][nm]) for nm in in_names]
    abstract = [gshape(a) for a in sample]
    del sample

    box = {}
    def compile_job():
        try:
            box["c"] = sharded.lower(*abstract).compile()
        except Exception as exc:  # noqa: BLE001
            box["e"] = exc
    th = threading.Thread(target=compile_job)
    th.start()

    # build global host arrays while the compile thread runs; the compiled
    # call transfers them (explicit device_put hits a pathological one-time
    # init on the axon PJRT plugin, so pass numpy directly)
    global_arrays = global_arrays or {}
    np_args = []
    for nm in in_names:
        if nm in global_arrays:
            np_args.append(np.ascontiguousarray(global_arrays[nm]))
        else:
            np_args.append(np.concatenate([np.asarray(m[nm]) for m in in_maps],
                                          axis=0))
    th.join()
    if "e" in box:
        raise box["e"]

    out_arrs = box["c"](*np_args)
    host = [np.asarray(a) for a in out_arrs]

    class _R:
        pass
    r = _R()
    r.results = [
        {nm: host[i].reshape(n_cores, *out_avals[i].shape)[c]
         for i, nm in enumerate(out_names)}
        for c in range(n_cores)
    ]
    return r


# ---------------------------------------------------------------- kernel
def _bf(x):
    return np.ascontiguousarray(x.astype(ml_dtypes.bfloat16))


def _waug_eff(W, a_s, a_d, b_prev):
    """Table row = [prev+b | al_s | al_d];  al = (prev+b) @ W_a."""
    Fin = W.shape[0]
    FHl = W.shape[1] // H
    Wal = np.einsum("ihf,hf->ih", W.reshape(Fin, H, FHl), a_s)
    Wad = np.einsum("ihf,hf->ih", W.reshape(Fin, H, FHl), a_d)
    wa = np.zeros((Fin + 1, TW), np.float32)
    wa[:Fin, :Fin] = np.eye(Fin, dtype=np.float32)
    wa[Fin, :Fin] = b_prev
    wa[:Fin, Fin:Fin + 8] = Wal
    wa[Fin, Fin:Fin + 8] = b_prev @ Wal
    wa[:Fin, Fin + 8:Fin + 16] = Wad
    wa[Fin, Fin + 8:Fin + 16] = b_prev @ Wad
    return wa


def kernel(**inputs):
    # warm the jax/axon backend while we build and prep
    def _warm():
        try:
            import jax
            jax.devices()
        except Exception:  # noqa: BLE001
            pass
    warm_th = threading.Thread(target=_warm)
    warm_th.start()

    x = np.asarray(inputs["x"], np.float32)
    ei = np.asarray(inputs["edge_index"])
    ea = np.asarray(inputs["edge_attr"], np.float32)
    cores = list(range(CORES))

    src = ei[0].astype(np.int64)
    dst = ei[1].astype(np.int64)
    loop = np.arange(N, dtype=np.int64)
    src_sl = np.concatenate([src, loop])
    dst_sl = np.concatenate([dst, loop])

    # dynamic SUB from actual chunk occupancy
    key = (dst_sl >> 7) * SUBS + src_sl // CH
    counts = np.bincount(key, minlength=TILES * SUBS)
    SUB = int(-(-counts.max() // 128) * 128)
    SLOTS = SUBS * SUB
    GROUPS = SLOTS // 128

    # build + trace + compile in a worker thread; host prep + transfers here
    build_box = {}
    def build_job():
        try:
            build_box["nc"] = build_fused(SUB)
        except Exception as exc:  # noqa: BLE001
            build_box["e"] = exc
    build_th = threading.Thread(target=build_job)
    build_th.start()

    idx_w, dl, slot_of = _sort_edges(src_sl, dst_sl, SUB)

    idx_g = idx_w.reshape(TILES * 16, -1)
    dl_g = dl.reshape(TILES * 128, -1)

    # attr in slot space, 10 cols, bf16
    attr_slot = np.zeros((TILES * SLOTS, 10), ml_dtypes.bfloat16)
    attr_slot[slot_of[:E]] = ea.astype(ml_dtypes.bfloat16)
    attr_g = np.ascontiguousarray(
        attr_slot.reshape(TILES, GROUPS, 128, 10).transpose(0, 2, 1, 3)
    ).reshape(TILES * 128, GROUPS * 10)

    # x shards: [4, OWN] per core (row 3 = ones)
    xT = np.zeros((4, NP), np.float32)
    xT[:3, :N] = x.T
    xT[3, :] = 1.0
    xT_sh = [np.ascontiguousarray(xT[:, cc * OWN:(cc + 1) * OWN]) for cc in cores]

    W1 = np.asarray(inputs["W1"], np.float32)
    W2g = np.asarray(inputs["W2"], np.float32)
    W3g = np.asarray(inputs["W3"], np.float32)
    b1 = np.asarray(inputs["b1"], np.float32)
    b2g = np.asarray(inputs["b2"], np.float32)
    b3 = np.asarray(inputs["b3"], np.float32)
    wa1 = _waug_eff(W1, np.asarray(inputs["as1"], np.float32),
                    np.asarray(inputs["ad1"], np.float32), np.zeros(3, np.float32))
    wa2 = _waug_eff(W2g, np.asarray(inputs["as2"], np.float32),
                    np.asarray(inputs["ad2"], np.float32), b1)
    wa3 = _waug_eff(W3g, np.asarray(inputs["as3"], np.float32),
                    np.asarray(inputs["ad3"], np.float32), b2g)
    # device epilogues emit head-SUM (not mean): fold 1/H into the rows that
    # multiply the previous layer's output
    wa2[:16] /= H
    wa3[:32] /= H

    Wm1 = np.asarray(inputs["Wm1"], np.float32)
    bm1 = np.asarray(inputs["bm1"], np.float32)
    Wm2 = np.asarray(inputs["Wm2"], np.float32)
    bm2 = np.asarray(inputs["bm2"], np.float32)
    Wm3 = np.asarray(inputs["Wm3"], np.float32)
    bm3 = np.asarray(inputs["bm3"], np.float32)
    Wu, Wv, Wc_ = Wm1[:64], Wm1[64:128], Wm1[128:138]
    wuv = np.zeros((65, 128), np.float32)
    wuv[:64, :64] = Wu / H
    wuv[64, :64] = b3 @ Wu + 0.5 * bm1
    wuv[:64, 64:] = Wv / H
    wuv[64, 64:] = b3 @ Wv + 0.5 * bm1
    w3p = np.zeros((16, 8), np.float32)
    w3p[:, 0:1] = Wm3

    in_maps = []
    for cc in cores:
        in_maps.append({
            "xT": xT_sh[cc], "wa1": wa1, "wa2": wa2, "wa3": wa3,
            "wm1": _bf(W1), "wm2": _bf(W2g), "wm3": _bf(W3g),
            "wuv": wuv, "wc": _bf(Wc_), "w2": _bf(Wm2),
            "b2": bm2.reshape(16, 1), "w3": _bf(w3p), "bm3": bm3.reshape(1, 1),
            "idx": idx_g[cc * TPC * 16:(cc + 1) * TPC * 16],
            "dloc": dl_g[cc * TPC * 128:(cc + 1) * TPC * 128],
            "attr": attr_g[cc * TPC * 128:(cc + 1) * TPC * 128],
        })
    global_arrays = {"idx": idx_g, "dloc": dl_g, "attr": attr_g}

    build_th.join()
    if "e" in build_box:
        raise build_box["e"]
    nc = build_box["nc"]
    warm_th.join()

    try:
        res = _run_custom(nc, in_maps, CORES, global_arrays)
    except Exception as exc:  # noqa: BLE001
        import traceback, sys as _sys
        print(f"custom runner failed ({exc!r}); falling back", file=_sys.stderr)
        traceback.print_exc()
        res = run_bass_kernel_spmd(nc, in_maps, cores)
    oslots = np.concatenate([r["out_slots"] for r in res.results], 0)
    oslots = oslots.reshape(-1).astype(np.float32)
    out = oslots[slot_of[:E]]
    return out.reshape(E, 1)


# revision 12
# speedup vs baseline: 11.2749x; 3.8808x over previous
"""Trainium2 Bass kernel for nn_BasicAttentionModel (3-layer GAT + edge MLP).

Fused single-launch design (8-core SPMD, dst-partitioned edges):
  - One Bass program holds all four phases (GAT x3 + edge MLP); intermediate
    node tables never leave the device.  Each core owns 98 consecutive
    128-node tiles (its dst range) and processes only its own edges; the
    per-layer node tables [prev+b | al_s | al_d] are computed in the edge
    phase epilogue (transpose + matmul with an augmented weight) and
    AllGathered across the 8 cores between phases.
  - Edges (with self-loops) are dst-sorted into tiles and grouped into 4
    src-chunk sub-tiles (int16 gather indices); the sub-tile size SUB is
    sized from the actual max chunk occupancy, rounded up to 128.
  - The edge MLP reuses the same slot layout (self-loop slots discarded on
    the host); edge_attr ships as 10-col bf16 in slot order; gather indices
    ship once (16 rows per tile) and are replicated across partitions on
    device.
  - The runner overlaps program build + jit trace + neuronx compile (worker
    thread) with host-side edge sorting and per-device input streaming
    (main thread).
"""
import threading
import numpy as np
import ml_dtypes

import concourse.bacc as bacc
import concourse.bass as bass
import concourse.mybir as mybir
import concourse.tile as tile
from concourse.bass_utils import run_bass_kernel_spmd
from concourse.masks import make_identity

F32 = mybir.dt.float32
BF16 = mybir.dt.bfloat16
I16 = mybir.dt.int16

N = 100000
E = 1600000
H = 8
CORES = 8
NP = 100352          # 784 * 128, divisible by 4 chunks of 25088
CH = 25088
TILE_N = 128
SUBS = 4
TILES = NP // TILE_N          # 784
TPC = TILES // CORES          # 98
OWN = TPC * TILE_N            # 12544 nodes per core
NODE_CH = 1792                # node-phase trip (OWN = 7 * 1792)
TW = 64                       # table row width (floats) = 256B


# ------------------------------------------------------------ host prep
def _sort_edges(src, dst, SUB):
    """Vectorized dst-tile / src-chunk slotting.
    Returns idx (int16, [TILES,16,SUBS*SUB/16]), dloc (f32
    [TILES,128,GROUPS]), slot_of (edge -> global slot)."""
    SLOTS = SUBS * SUB
    GROUPS = SLOTS // 128
    tile_i = (dst >> 7).astype(np.int64)
    chunk = src // CH
    key = tile_i * SUBS + chunk
    order = np.argsort(key, kind="stable")
    ks = key[order]
    starts = np.searchsorted(ks, np.arange(TILES * SUBS))
    rank = np.arange(len(ks)) - starts[ks]
    slot_sorted = tile_i[order] * SLOTS + chunk[order] * SUB + rank
    slot_of = np.empty(len(ks), np.int64)
    slot_of[order] = slot_sorted

    idx_flat = np.zeros(TILES * SLOTS, np.int16)
    idx_flat[slot_sorted] = (src[order] - chunk[order] * CH).astype(np.int16)
    dloc_flat = np.full(TILES * SLOTS, 255, np.int16)
    dloc_flat[slot_sorted] = (dst[order] - tile_i[order] * TILE_N).astype(np.int16)

    # wrap idx for dma_gather: j -> partition j%16, col j//16 (16 rows/tile;
    # replication to 128 partitions happens on device)
    w = idx_flat.reshape(TILES, SUBS, SUB // 16, 16)
    idx_w = np.transpose(w, (0, 3, 1, 2)).reshape(TILES, 16, SUBS * (SUB // 16))
    # dloc arranged [TILES, 128, GROUPS]: slot = g*128+p
    dl = dloc_flat.reshape(TILES, GROUPS, 128).transpose(0, 2, 1).copy()
    return idx_w, dl, slot_of


# ------------------------------------------------------------ program
def build_fused(SUB, n_swdge=1):
    SLOTS = SUBS * SUB
    GROUPS = SLOTS // 128
    spg = SUB // 128
    ICOL = SUBS * (SUB // 16)

    nc = bacc.Bacc("TRN2", target_bir_lowering=False, debug=False,
                   dynamic_dma_scratch_size=131072, num_swdge_queues=n_swdge,
                   num_devices=CORES)
    xT = nc.dram_tensor("xT", [4, OWN], F32, kind="ExternalInput")
    wa1_t = nc.dram_tensor("wa1", [4, TW], F32, kind="ExternalInput")
    wa2_t = nc.dram_tensor("wa2", [17, TW], F32, kind="ExternalInput")
    wa3_t = nc.dram_tensor("wa3", [33, TW], F32, kind="ExternalInput")
    wm1_t = nc.dram_tensor("wm1", [3, 128], BF16, kind="ExternalInput")
    wm2_t = nc.dram_tensor("wm2", [16, 256], BF16, kind="ExternalInput")
    wm3_t = nc.dram_tensor("wm3", [32, 512], BF16, kind="ExternalInput")
    wuv_t = nc.dram_tensor("wuv", [65, 128], F32, kind="ExternalInput")
    wc_t = nc.dram_tensor("wc", [10, 64], BF16, kind="ExternalInput")
    w2_t = nc.dram_tensor("w2", [64, 16], BF16, kind="ExternalInput")
    b2_t = nc.dram_tensor("b2", [16, 1], F32, kind="ExternalInput")
    w3_t = nc.dram_tensor("w3", [16, 8], BF16, kind="ExternalInput")
    b3_t = nc.dram_tensor("bm3", [1, 1], F32, kind="ExternalInput")
    idx_t = nc.dram_tensor("idx", [TPC * 16, ICOL], I16, kind="ExternalInput")
    dloc_t = nc.dram_tensor("dloc", [TPC * 128, GROUPS], I16, kind="ExternalInput")
    attr_t = nc.dram_tensor("attr", [TPC * 128, GROUPS * 10], BF16,
                            kind="ExternalInput")
    out_t = nc.dram_tensor("out_slots", [TPC, SLOTS], BF16, kind="ExternalOutput")

    own = [nc.dram_tensor(f"own{l}", [OWN, TW], F32) for l in range(3)]
    gtbl = [nc.dram_tensor(f"gtbl{l}", [NP, TW], F32) for l in range(3)]
    ownu = nc.dram_tensor("ownu", [OWN, TW], F32)
    ownv = nc.dram_tensor("ownv", [OWN, TW], F32)
    utbl = nc.dram_tensor("utbl", [NP, TW], F32)
    vtbl = nc.dram_tensor("vtbl", [NP, TW], F32)

    def allgather(src_t, dst_t):
        tc.strict_bb_all_engine_barrier()
        nc.gpsimd.collective_compute(
            "AllGather", mybir.AluOpType.bypass,
            replica_groups=[list(range(CORES))],
            ins=[src_t[:].opt()], outs=[dst_t[:].opt()])
        tc.strict_bb_all_engine_barrier()

    def load_idxs(psb, i):
        idxs = psb.tile([128, ICOL], I16)
        for k in range(8):
            nc.sync.dma_start(out=idxs[16 * k:16 * (k + 1), :],
                              in_=idx_t[bass.ts(i, 16), :])
        return idxs

    with tile.TileContext(nc) as tc:
        with tc.tile_pool(name="const", bufs=1) as cpool:
            wa1 = cpool.tile([4, TW], F32)
            nc.sync.dma_start(out=wa1[:], in_=wa1_t[:])
            wa2 = cpool.tile([17, TW], F32)
            nc.sync.dma_start(out=wa2[:], in_=wa2_t[:])
            wa3 = cpool.tile([33, TW], F32)
            nc.sync.dma_start(out=wa3[:], in_=wa3_t[:])
            wm1 = cpool.tile([3, 128], BF16)
            nc.sync.dma_start(out=wm1[:], in_=wm1_t[:])
            wm2 = cpool.tile([16, 256], BF16)
            nc.sync.dma_start(out=wm2[:], in_=wm2_t[:])
            wm3 = cpool.tile([32, 512], BF16)
            nc.sync.dma_start(out=wm3[:], in_=wm3_t[:])
            wuv = cpool.tile([65, 128], F32)
            nc.sync.dma_start(out=wuv[:], in_=wuv_t[:])
            wa2r = cpool.tile([1, TW], F32)
            nc.sync.dma_start(out=wa2r[:], in_=wa2_t[16:17, :])
            wa3r = cpool.tile([1, TW], F32)
            nc.sync.dma_start(out=wa3r[:], in_=wa3_t[32:33, :])
            wuvr = cpool.tile([1, 128], F32)
            nc.sync.dma_start(out=wuvr[:], in_=wuv_t[64:65, :])
            wc = cpool.tile([10, 64], BF16)
            nc.sync.dma_start(out=wc[:], in_=wc_t[:])
            w2 = cpool.tile([64, 16], BF16)
            nc.sync.dma_start(out=w2[:], in_=w2_t[:])
            b2s = cpool.tile([16, 1], F32)
            nc.sync.dma_start(out=b2s[:], in_=b2_t[:])
            w3 = cpool.tile([16, 8], BF16)
            nc.sync.dma_start(out=w3[:], in_=w3_t[:])
            b3s = cpool.tile([1, 1], F32)
            nc.sync.dma_start(out=b3s[:], in_=b3_t[:])
            iota = cpool.tile([128, 128], F32)
            nc.gpsimd.iota(iota[:], [[1, 128]], channel_multiplier=0,
                           allow_small_or_imprecise_dtypes=True)
            ident = cpool.tile([128, 128], F32)
            make_identity(nc, ident[:])
            identb = cpool.tile([128, 128], BF16)
            nc.vector.tensor_copy(out=identb[:], in_=ident[:])
            ones_r = cpool.tile([1, 128], F32)
            nc.vector.memset(ones_r[:], 1.0)

            pid = nc.sync.partition_id()

            # ------------- phase 0: layer-1 node table from own x shard
            with tc.tile_pool(name="n_in", bufs=2) as pin, \
                 tc.tile_pool(name="n_out", bufs=2) as pout, \
                 tc.tile_pool(name="n_ps", bufs=2, space="PSUM") as pps:
                with tc.For_i(0, OWN // NODE_CH, 1) as i:
                    pv = pin.tile([4, NODE_CH], F32)
                    nc.sync.dma_start(out=pv[:], in_=xT[:, bass.ts(i, NODE_CH)])
                    ob = pout.tile([128, NODE_CH // 128, TW], F32)
                    for k in range(NODE_CH // 128):
                        ps = pps.tile([128, TW], F32, space="PSUM")
                        nc.tensor.matmul(out=ps[:], lhsT=pv[:, k * 128:(k + 1) * 128],
                                         rhs=wa1[:], start=True, stop=True)
                        nc.scalar.copy(out=ob[:, k, :], in_=ps[:])
                    nc.sync.dma_start(
                        out=own[0][bass.ts(i, NODE_CH), :].rearrange(
                            "(k p) w -> p k w", p=128),
                        in_=ob[:])
            allgather(own[0], gtbl[0])

            # ------------- GAT edge phases
            layer_cfg = [
                (3, 128, 16, wm1, wa2, wa2r),
                (16, 256, 32, wm2, wa3, wa3r),
                (32, 512, 64, wm3, None, None),
            ]
            for l, (F_in, HF, F_out, wm, wa_next, wa_bias) in enumerate(layer_cfg):
                FH = HF // H
                gt_l = gtbl[l]
                with tc.tile_pool(name=f"eg{l}", bufs=2) as pg, \
                     tc.tile_pool(name=f"es{l}", bufs=2) as psb, \
                     tc.tile_pool(name=f"eps{l}", bufs=1, space="PSUM") as pps, \
                     tc.tile_pool(name=f"eac{l}", bufs=1, space="PSUM") as pac:
                    with tc.For_i(0, TPC, 1) as i:
                        idxs = load_idxs(psb, i)
                        dlc16 = psb.tile([128, GROUPS], I16)
                        nc.sync.dma_start(out=dlc16[:], in_=dloc_t[bass.ts(i, 128), :])
                        dlc = psb.tile([128, GROUPS], F32)
                        nc.vector.tensor_copy(out=dlc[:], in_=dlc16[:])
                        adn = psb.tile([128, 8], F32)
                        nc.sync.dma_start(
                            out=adn[:],
                            in_=gt_l[bass.ds((pid * TPC + i) * 128, 128),
                                     F_in + 8:F_in + 16])
                        adn_bf = psb.tile([128, 8], BF16)
                        nc.scalar.copy(out=adn_bf[:], in_=adn[:])

                        gt = pg.tile([128, GROUPS, TW], F32)
                        for s in range(SUBS):
                            nc.gpsimd.dma_gather(
                                out_ap=gt[:, s * spg:(s + 1) * spg, :],
                                in_ap=gt_l[s * CH:(s + 1) * CH, :],
                                idxs_ap=idxs[:, s * (SUB // 16):(s + 1) * (SUB // 16)],
                                num_idxs=SUB, num_idxs_reg=SUB,
                                elem_size=TW, single_packet=False,
                                queue_num=s % n_swdge)

                        vex = pg.tile([128, GROUPS, HF], BF16)
                        exb = psb.tile([128, GROUPS, H], BF16)
                        acc = pac.tile([128, HF], F32, space="PSUM")
                        den = pac.tile([128, H], F32, space="PSUM")
                        for g in range(GROUPS):
                            st = psb.tile([128, 128], BF16, tag="st")
                            nc.vector.tensor_scalar(
                                out=st[:], in0=iota[:], scalar1=dlc[:, g:g + 1],
                                scalar2=None, op0=mybir.AluOpType.is_equal)
                            tp = pps.tile([F_in, 128], F32, space="PSUM", tag="tp")
                            nc.tensor.transpose(out=tp[:], in_=gt[:, g, 0:F_in],
                                                identity=ident[:])
                            tpb = psb.tile([F_in, 128], BF16, tag="tpb")
                            nc.scalar.copy(out=tpb[:], in_=tp[:])
                            hp = pps.tile([128, HF], F32, space="PSUM", tag="hp")
                            nc.tensor.matmul(out=hp[:], lhsT=tpb[:], rhs=wm[:],
                                             start=True, stop=True)
                            sg = pps.tile([128, 128], BF16, space="PSUM", tag="sg")
                            nc.tensor.transpose(out=sg[:], in_=st[:],
                                                identity=identb[:])
                            sgb = psb.tile([128, 128], BF16, tag="sgb")
                            nc.scalar.copy(out=sgb[:], in_=sg[:])
                            ep = pps.tile([128, H], F32, space="PSUM", tag="ep")
                            nc.tensor.matmul(out=ep[:], lhsT=sgb[:], rhs=adn_bf[:],
                                             start=True, stop=True)
                            ef = psb.tile([128, H], F32, tag="ef")
                            nc.vector.tensor_add(out=ef[:], in0=ep[:],
                                                 in1=gt[:, g, F_in:F_in + 8])
                            eft = psb.tile([128, H], F32, tag="eft")
                            nc.vector.tensor_scalar(
                                out=eft[:], in0=ef[:], scalar1=0.2, scalar2=None,
                                op0=mybir.AluOpType.mult)
                            nc.vector.tensor_tensor(
                                out=ef[:], in0=ef[:], in1=eft[:],
                                op=mybir.AluOpType.max)
                            exf = psb.tile([128, H], F32, tag="exf")
                            nc.scalar.activation(out=exf[:], in_=ef[:],
                                                 func=mybir.ActivationFunctionType.Exp)
                            nc.vector.tensor_copy(out=exb[:, g, :], in_=exf[:])
                            for h in range(H):
                                if h % 2 == 0:
                                    nc.vector.tensor_scalar(
                                        out=vex[:, g, h * FH:(h + 1) * FH],
                                        in0=hp[:, h * FH:(h + 1) * FH],
                                        scalar1=exf[:, h:h + 1], scalar2=None,
                                        op0=mybir.AluOpType.mult)
                                else:
                                    nc.scalar.activation(
                                        out=vex[:, g, h * FH:(h + 1) * FH],
                                        in_=hp[:, h * FH:(h + 1) * FH],
                                        func=mybir.ActivationFunctionType.Copy,
                                        scale=exf[:, h:h + 1])
                            nc.tensor.matmul(out=acc[:], lhsT=st[:], rhs=vex[:, g, :],
                                             start=(g == 0), stop=(g == GROUPS - 1))
                            nc.tensor.matmul(out=den[:], lhsT=st[:], rhs=exb[:, g, :],
                                             start=(g == 0), stop=(g == GROUPS - 1))
                        # epilogue: out = sum_h acc_h / den_h (1/H folded into
                        # the next-layer weights on host; bias deferred)
                        dr = psb.tile([128, H], F32, tag="dr")
                        nc.vector.tensor_scalar(
                            out=dr[:], in0=den[:], scalar1=1e-30, scalar2=None,
                            op0=mybir.AluOpType.add)
                        nc.vector.reciprocal(out=dr[:], in_=dr[:])
                        ot = psb.tile([128, H, F_out], F32, tag="ot")
                        for h in range(H):
                            if h % 2 == 0:
                                nc.vector.tensor_scalar(
                                    out=ot[:, h, :], in0=acc[:, h * FH:(h + 1) * FH],
                                    scalar1=dr[:, h:h + 1], scalar2=None,
                                    op0=mybir.AluOpType.mult)
                            else:
                                nc.scalar.activation(
                                    out=ot[:, h, :], in_=acc[:, h * FH:(h + 1) * FH],
                                    func=mybir.ActivationFunctionType.Copy,
                                    scale=dr[:, h:h + 1])
                        for step in [4, 2, 1]:
                            for h in range(step):
                                nc.vector.tensor_add(out=ot[:, h, :], in0=ot[:, h, :],
                                                     in1=ot[:, h + step, :])
                        # fused next-table epilogue
                        tpo = pps.tile([F_out, 128], F32, space="PSUM", tag="tpo")
                        nc.tensor.transpose(out=tpo[:], in_=ot[:, 0, :],
                                            identity=ident[:])
                        tpo_s = psb.tile([F_out, 128], F32, tag="tpos")
                        nc.scalar.copy(out=tpo_s[:], in_=tpo[:])
                        if l < 2:
                            nt = pps.tile([128, TW], F32, space="PSUM", tag="nt")
                            nc.tensor.matmul(out=nt[:], lhsT=tpo_s[:],
                                             rhs=wa_next[0:F_out, :],
                                             start=True, stop=False)
                            nc.tensor.matmul(out=nt[:], lhsT=ones_r[:],
                                             rhs=wa_bias[:],
                                             start=False, stop=True)
                            nts = psb.tile([128, TW], F32, tag="nts")
                            nc.scalar.copy(out=nts[:], in_=nt[:])
                            nc.sync.dma_start(out=own[l + 1][bass.ts(i, 128), :],
                                              in_=nts[:])
                        else:
                            nt = pps.tile([128, 128], F32, space="PSUM", tag="nt")
                            nc.tensor.matmul(out=nt[:], lhsT=tpo_s[:],
                                             rhs=wuv[0:64, :],
                                             start=True, stop=False)
                            nc.tensor.matmul(out=nt[:], lhsT=ones_r[:],
                                             rhs=wuvr[:],
                                             start=False, stop=True)
                            nts = psb.tile([128, 128], F32, tag="nts")
                            nc.scalar.copy(out=nts[:], in_=nt[:])
                            nc.sync.dma_start(out=ownu[bass.ts(i, 128), :],
                                              in_=nts[:, 0:64])
                            nc.sync.dma_start(out=ownv[bass.ts(i, 128), :],
                                              in_=nts[:, 64:128])
                if l < 2:
                    allgather(own[l + 1], gtbl[l + 1])
                else:
                    allgather(ownu, utbl)
                    allgather(ownv, vtbl)

            # ------------- phase 4: edge MLP
            with tc.tile_pool(name="mg", bufs=2) as pg, \
                 tc.tile_pool(name="ms", bufs=2) as psb, \
                 tc.tile_pool(name="mps", bufs=1, space="PSUM") as pps:
                with tc.For_i(0, TPC, 1) as i:
                    idxs = load_idxs(psb, i)
                    dlc16 = psb.tile([128, GROUPS], I16)
                    nc.sync.dma_start(out=dlc16[:], in_=dloc_t[bass.ts(i, 128), :])
                    dlc = psb.tile([128, GROUPS], F32)
                    nc.vector.tensor_copy(out=dlc[:], in_=dlc16[:])
                    vnd = psb.tile([128, 64], F32)
                    nc.sync.dma_start(
                        out=vnd[:],
                        in_=vtbl[bass.ds((pid * TPC + i) * 128, 128), :])
                    vnd_bf = psb.tile([128, 64], BF16)
                    nc.scalar.copy(out=vnd_bf[:], in_=vnd[:])
                    att = pg.tile([128, GROUPS, 10], BF16, tag="att")
                    nc.sync.dma_start(out=att[:].rearrange("p g w -> p (g w)"),
                                      in_=attr_t[bass.ts(i, 128), :])

                    gt = pg.tile([128, GROUPS, 64], F32)
                    for s in range(SUBS):
                        nc.gpsimd.dma_gather(
                            out_ap=gt[:, s * spg:(s + 1) * spg, :],
                            in_ap=utbl[s * CH:(s + 1) * CH, :],
                            idxs_ap=idxs[:, s * (SUB // 16):(s + 1) * (SUB // 16)],
                            num_idxs=SUB, num_idxs_reg=SUB,
                            elem_size=64, single_packet=False,
                            queue_num=s % n_swdge)

                    orow = psb.tile([1, GROUPS, 128], BF16, tag="orow")
                    for g in range(GROUPS):
                        st = psb.tile([128, 128], BF16, tag="st")
                        nc.vector.tensor_scalar(
                            out=st[:], in0=iota[:], scalar1=dlc[:, g:g + 1],
                            scalar2=None, op0=mybir.AluOpType.is_equal)
                        sg = pps.tile([128, 128], BF16, space="PSUM", tag="sg")
                        nc.tensor.transpose(out=sg[:], in_=st[:], identity=identb[:])
                        sgb = psb.tile([128, 128], BF16, tag="sgb")
                        nc.scalar.copy(out=sgb[:], in_=sg[:])
                        atp = pps.tile([10, 128], BF16, space="PSUM", tag="atp")
                        nc.tensor.transpose(out=atp[:], in_=att[:, g, :],
                                            identity=identb[:])
                        atpb = psb.tile([10, 128], BF16, tag="atpb")
                        nc.scalar.copy(out=atpb[:], in_=atp[:])
                        z1p = pps.tile([128, 64], F32, space="PSUM", tag="z1p")
                        nc.tensor.matmul(out=z1p[:], lhsT=atpb[:], rhs=wc[:],
                                         start=True, stop=False)
                        nc.tensor.matmul(out=z1p[:], lhsT=sgb[:], rhs=vnd_bf[:],
                                         start=False, stop=True)
                        z1 = psb.tile([128, 64], F32, tag="z1")
                        nc.vector.tensor_add(out=z1[:], in0=z1p[:], in1=gt[:, g, :])
                        z1s = psb.tile([128, 64], F32, tag="z1s")
                        nc.vector.tensor_scalar(
                            out=z1s[:], in0=z1[:], scalar1=0.12, scalar2=None,
                            op0=mybir.AluOpType.mult)
                        z1b = psb.tile([128, 64], BF16, tag="z1b")
                        nc.vector.tensor_tensor(
                            out=z1b[:], in0=z1[:], in1=z1s[:],
                            op=mybir.AluOpType.max)
                        z1t = pps.tile([64, 128], BF16, space="PSUM", tag="z1t")
                        nc.tensor.transpose(out=z1t[:], in_=z1b[:], identity=identb[:])
                        z1tb = psb.tile([64, 128], BF16, tag="z1tb")
                        nc.scalar.copy(out=z1tb[:], in_=z1t[:])
                        z2p = pps.tile([16, 128], F32, space="PSUM", tag="z2p")
                        nc.tensor.matmul(out=z2p[:], lhsT=w2[:], rhs=z1tb[:],
                                         start=True, stop=True)
                        z2f = psb.tile([16, 128], F32, tag="z2f")
                        nc.vector.tensor_scalar(
                            out=z2f[:], in0=z2p[:], scalar1=b2s[:, 0:1], scalar2=None,
                            op0=mybir.AluOpType.add)
                        z2s = psb.tile([16, 128], F32, tag="z2s")
                        nc.vector.tensor_scalar(
                            out=z2s[:], in0=z2f[:], scalar1=0.12, scalar2=None,
                            op0=mybir.AluOpType.mult)
                        z2b = psb.tile([16, 128], BF16, tag="z2b")
                        nc.vector.tensor_tensor(
                            out=z2b[:], in0=z2f[:], in1=z2s[:],
                            op=mybir.AluOpType.max)
                        z3p = pps.tile([8, 128], F32, space="PSUM", tag="z3p")
                        nc.tensor.matmul(out=z3p[:], lhsT=w3[:], rhs=z2b[:],
                                         start=True, stop=True)
                        nc.scalar.activation(out=orow[:, g, :], in_=z3p[0:1, :],
                                             func=mybir.ActivationFunctionType.Sigmoid,
                                             bias=b3s[:, 0:1])
                    nc.sync.dma_start(
                        out=out_t[bass.ts(i, 1), :],
                        in_=orow[:].rearrange("o g p -> o (g p)"))
    nc.compile()
    return nc


# ------------------------------------------------------------ runner
def _run_custom(nc, in_maps, n_cores, global_arrays=None):
    import jax
    from jax.sharding import Mesh, PartitionSpec, NamedSharding
    try:
        from jax.experimental.shard_map import shard_map
    except ImportError:
        from jax.shard_map import shard_map
    from concourse import bass2jax

    bass2jax.install_neuronx_cc_hook()
    partition_name = nc.partition_id_tensor.name if nc.partition_id_tensor else None
    in_names, out_names, out_avals = [], [], []
    for alloc in nc.m.functions[0].allocations:
        if not isinstance(alloc, mybir.MemoryLocationSet):
            continue
        name = alloc.memorylocations[0].name
        if alloc.kind == "ExternalInput":
            if name != partition_name:
                in_names.append(name)
        elif alloc.kind == "ExternalOutput":
            out_names.append(name)
            shape = tuple(alloc.tensor_shape)
            dtype = mybir.dt.np(alloc.dtype)
            out_avals.append(jax.core.ShapedArray(shape, dtype))
    n_params = len(in_names)
    n_outs = len(out_avals)
    all_in_names = list(in_names)
    if partition_name is not None:
        all_in_names.append(partition_name)

    def _body(*args):
        operands = list(args)
        if partition_name is not None:
            operands.append(bass2jax.partition_id_tensor())
        outs = bass2jax._bass_exec_p.bind(
            *operands,
            out_avals=tuple(out_avals),
            in_names=tuple(all_in_names),
            out_names=tuple(out_names),
            lowering_input_output_aliases=(),
            sim_require_finite=True,
            sim_require_nnan=True,
            nc=nc,
        )
        return tuple(outs)

    devices = jax.devices()[:n_cores]
    mesh = Mesh(np.asarray(devices), ("core",))
    sharding = NamedSharding(mesh, PartitionSpec("core"))
    in_specs = (PartitionSpec("core"),) * n_params
    out_specs = (PartitionSpec("core"),) * n_outs
    sharded = jax.jit(
        shard_map(_body, mesh=mesh, in_specs=in_specs, out_specs=out_specs,
                  check_rep=False),
        keep_unused=True)

    def gshape(a):
        return jax.ShapeDtypeStruct((n_cores * a.shape[0], *a.shape[1:]), a.dtype,
                                    sharding=sharding)
    sample = [np.asarray(in_maps[0][nm]) for nm in in_names]
    abstract = [gshape(a) for a in sample]
    del sample

    box = {}
    def compile_job():
        try:
            box["c"] = sharded.lower(*abstract).compile()
        except Exception as exc:  # noqa: BLE001
            box["e"] = exc
    th = threading.Thread(target=compile_job)
    th.start()

    # build global host arrays while the compile thread runs; the compiled
    # call transfers them (explicit device_put hits a pathological one-time
    # init on the axon PJRT plugin, so pass numpy directly)
    global_arrays = global_arrays or {}
    np_args = []
    for nm in in_names:
        if nm in global_arrays:
            np_args.append(np.ascontiguousarray(global_arrays[nm]))
        else:
            np_args.append(np.concatenate([np.asarray(m[nm]) for m in in_maps],
                                          axis=0))
    th.join()
    if "e" in box:
        raise box["e"]

    out_arrs = box["c"](*np_args)
    host = [np.asarray(a) for a in out_arrs]

    class _R:
        pass
    r = _R()
    r.results = [
        {nm: host[i].reshape(n_cores, *out_avals[i].shape)[c]
         for i, nm in enumerate(out_names)}
        for c in range(n_cores)
    ]
    return r


# ---------------------------------------------------------------- kernel
def _bf(x):
    return np.ascontiguousarray(x.astype(ml_dtypes.bfloat16))


def _waug_eff(W, a_s, a_d, b_prev):
    """Table row = [prev+b | al_s | al_d];  al = (prev+b) @ W_a."""
    Fin = W.shape[0]
    FHl = W.shape[1] // H
    Wal = np.einsum("ihf,hf->ih", W.reshape(Fin, H, FHl), a_s)
    Wad = np.einsum("ihf,hf->ih", W.reshape(Fin, H, FHl), a_d)
    wa = np.zeros((Fin + 1, TW), np.float32)
    wa[:Fin, :Fin] = np.eye(Fin, dtype=np.float32)
    wa[Fin, :Fin] = b_prev
    wa[:Fin, Fin:Fin + 8] = Wal
    wa[Fin, Fin:Fin + 8] = b_prev @ Wal
    wa[:Fin, Fin + 8:Fin + 16] = Wad
    wa[Fin, Fin + 8:Fin + 16] = b_prev @ Wad
    return wa


def kernel(**inputs):
    # warm the jax/axon backend while we build and prep
    def _warm():
        try:
            import jax
            jax.devices()
        except Exception:  # noqa: BLE001
            pass
    warm_th = threading.Thread(target=_warm)
    warm_th.start()

    x = np.asarray(inputs["x"], np.float32)
    ei = np.asarray(inputs["edge_index"])
    ea = np.asarray(inputs["edge_attr"], np.float32)
    cores = list(range(CORES))

    src = ei[0].astype(np.int64)
    dst = ei[1].astype(np.int64)
    loop = np.arange(N, dtype=np.int64)
    src_sl = np.concatenate([src, loop])
    dst_sl = np.concatenate([dst, loop])

    # dynamic SUB from actual chunk occupancy
    key = (dst_sl >> 7) * SUBS + src_sl // CH
    counts = np.bincount(key, minlength=TILES * SUBS)
    SUB = int(-(-counts.max() // 128) * 128)
    SLOTS = SUBS * SUB
    GROUPS = SLOTS // 128

    # build + trace + compile in a worker thread; host prep + transfers here
    build_box = {}
    def build_job():
        try:
            build_box["nc"] = build_fused(SUB)
        except Exception as exc:  # noqa: BLE001
            build_box["e"] = exc
    build_th = threading.Thread(target=build_job)
    build_th.start()

    idx_w, dl, slot_of = _sort_edges(src_sl, dst_sl, SUB)

    idx_g = idx_w.reshape(TILES * 16, -1)
    dl_g = dl.reshape(TILES * 128, -1)

    # attr in slot space, 10 cols, bf16
    attr_slot = np.zeros((TILES * SLOTS, 10), ml_dtypes.bfloat16)
    attr_slot[slot_of[:E]] = ea.astype(ml_dtypes.bfloat16)
    attr_g = np.ascontiguousarray(
        attr_slot.reshape(TILES, GROUPS, 128, 10).transpose(0, 2, 1, 3)
    ).reshape(TILES * 128, GROUPS * 10)

    # x shards: [4, OWN] per core (row 3 = ones)
    xT = np.zeros((4, NP), np.float32)
    xT[:3, :N] = x.T
    xT[3, :] = 1.0
    xT_sh = [np.ascontiguousarray(xT[:, cc * OWN:(cc + 1) * OWN]) for cc in cores]

    W1 = np.asarray(inputs["W1"], np.float32)
    W2g = np.asarray(inputs["W2"], np.float32)
    W3g = np.asarray(inputs["W3"], np.float32)
    b1 = np.asarray(inputs["b1"], np.float32)
    b2g = np.asarray(inputs["b2"], np.float32)
    b3 = np.asarray(inputs["b3"], np.float32)
    wa1 = _waug_eff(W1, np.asarray(inputs["as1"], np.float32),
                    np.asarray(inputs["ad1"], np.float32), np.zeros(3, np.float32))
    wa2 = _waug_eff(W2g, np.asarray(inputs["as2"], np.float32),
                    np.asarray(inputs["ad2"], np.float32), b1)
    wa3 = _waug_eff(W3g, np.asarray(inputs["as3"], np.float32),
                    np.asarray(inputs["ad3"], np.float32), b2g)
    # device epilogues emit head-SUM (not mean): fold 1/H into the rows that
    # multiply the previous layer's output
    wa2[:16] /= H
    wa3[:32] /= H

    Wm1 = np.asarray(inputs["Wm1"], np.float32)
    bm1 = np.asarray(inputs["bm1"], np.float32)
    Wm2 = np.asarray(inputs["Wm2"], np.float32)
    bm2 = np.asarray(inputs["bm2"], np.float32)
    Wm3 = np.asarray(inputs["Wm3"], np.float32)
    bm3 = np.asarray(inputs["bm3"], np.float32)
    Wu, Wv, Wc_ = Wm1[:64], Wm1[64:128], Wm1[128:138]
    wuv = np.zeros((65, 128), np.float32)
    wuv[:64, :64] = Wu / H
    wuv[64, :64] = b3 @ Wu + 0.5 * bm1
    wuv[:64, 64:] = Wv / H
    wuv[64, 64:] = b3 @ Wv + 0.5 * bm1
    w3p = np.zeros((16, 8), np.float32)
    w3p[:, 0:1] = Wm3

    in_maps = []
    for cc in cores:
        in_maps.append({
            "xT": xT_sh[cc], "wa1": wa1, "wa2": wa2, "wa3": wa3,
            "wm1": _bf(W1), "wm2": _bf(W2g), "wm3": _bf(W3g),
            "wuv": wuv, "wc": _bf(Wc_), "w2": _bf(Wm2),
            "b2": bm2.reshape(16, 1), "w3": _bf(w3p), "bm3": bm3.reshape(1, 1),
            "idx": idx_g[cc * TPC * 16:(cc + 1) * TPC * 16],
            "dloc": dl_g[cc * TPC * 128:(cc + 1) * TPC * 128],
            "attr": attr_g[cc * TPC * 128:(cc + 1) * TPC * 128],
        })
    global_arrays = {"idx": idx_g, "dloc": dl_g, "attr": attr_g}

    build_th.join()
    if "e" in build_box:
        raise build_box["e"]
    nc = build_box["nc"]
    warm_th.join()

    try:
        res = _run_custom(nc, in_maps, CORES, global_arrays)
    except Exception as exc:  # noqa: BLE001
        import traceback, sys as _sys
        print(f"custom runner failed ({exc!r}); falling back", file=_sys.stderr)
        traceback.print_exc()
        res = run_bass_kernel_spmd(nc, in_maps, cores)
    oslots = np.concatenate([r["out_slots"] for r in res.results], 0)
    oslots = oslots.reshape(-1).astype(np.float32)
    out = oslots[slot_of[:E]]
    return out.reshape(E, 1)
